# revision 6
# baseline (speedup 1.0000x reference)
# Trainium2 Bass kernel for nn_EARLIEST (adaptive-halting LSTM, B=128 T=4096
# V=128 H=256 C=10).
#
# Key observation: the model halts each batch sample at the first step t where
# u[b,t] < probs[b,t], with probs ~= 0.45 early on, so every sample halts
# within a few dozen steps (max 36 for the seed-0 inputs).  The returned
# output only needs logits at each sample's first halt step (or step T-1 for
# never-halted samples).  So the device kernel runs the LSTM scan for only
# T_EFF timesteps, emits pre-softmax logits and the halting dot-product for
# every (t, b), and the host applies the (exact) halting latch.  A numpy
# fallback continues the recurrence from the device's (h, c) state for any
# sample that has not halted by T_EFF (statistically never happens; the
# fallback keeps the kernel correct for arbitrary inputs).
#
# Sharding: data-parallel over batch, 16 samples per core, weights replicated.
# Layout on device is feature-major: h^T is [H=256, b=16] stored as two
# 128-partition k-tiles side by side, so LSTM gate math runs on full
# 128-partition tiles and the recurrent matmuls need no transposes.

import numpy as np

import concourse.bass as bass
import concourse.mybir as mybir
from concourse.bass_utils import run_bass_kernel_spmd

B, T_FULL, V, H, C = 128, 4096, 128, 256, 10
EPS = 0.1
NCORES = 8
BL = B // NCORES  # 16 samples per core
T_EFF = 64
M_TILES = 8   # 4H/128
K2 = 2        # H/128
F32 = mybir.dt.float32

# gate order in z is (i, f, g, o); we permute weight columns to (i, f, o, g)
# so one sigmoid covers a contiguous [i|f|o] block and one tanh covers g.
GATE_PERM = np.concatenate([np.arange(0, 512), np.arange(768, 1024),
                            np.arange(512, 768)])


def _build(T):
    """Build the raw-bass single-core program (SPMD across 8 cores)."""
    nc = bass.Bass()

    d_Xt = nc.dram_tensor("Xt", [128, T * BL], F32, kind="ExternalInput")
    d_WkT = nc.dram_tensor("WkT", [128, 1024], F32, kind="ExternalInput")
    d_WrT = nc.dram_tensor("WrT", [128, 2048], F32, kind="ExternalInput")
    d_blstm = nc.dram_tensor("blstm", [128, 8], F32, kind="ExternalInput")
    d_WoC = nc.dram_tensor("WoC", [128, 22], F32, kind="ExternalInput")
    d_bob = nc.dram_tensor("bob", [11, 1], F32, kind="ExternalInput")
    d_head = nc.dram_tensor("head", [11, T * BL], F32, kind="ExternalOutput")
    d_state = nc.dram_tensor("state", [128, 64], F32, kind="ExternalOutput")

    NH = T * BL  # head columns
    HALF = NH // 2  # 512 per fp32-matmul moving limit

    from contextlib import ExitStack
    ctx = ExitStack()
    sb_Xt = ctx.enter_context(nc.sbuf_tensor([128, T * BL], F32))
    sb_WkT = ctx.enter_context(nc.sbuf_tensor([128, 1024], F32))
    sb_WrT = ctx.enter_context(nc.sbuf_tensor([128, 2048], F32))
    sb_blstm = ctx.enter_context(nc.sbuf_tensor([128, 8], F32))
    sb_WoC = ctx.enter_context(nc.sbuf_tensor([128, 22], F32))
    sb_bob = ctx.enter_context(nc.sbuf_tensor([11, 1], F32))
    sb_XW = ctx.enter_context(nc.sbuf_tensor([128, T * 128], F32))
    sb_H = ctx.enter_context(nc.sbuf_tensor([128, (T + 1) * 32], F32))
    sb_C = ctx.enter_context(nc.sbuf_tensor([128, (T + 1) * 32], F32))
    sb_Z = ctx.enter_context(nc.sbuf_tensor([128, 2 * 128], F32))
    sb_G = ctx.enter_context(nc.sbuf_tensor([128, 2 * 128], F32))
    sb_TC = ctx.enter_context(nc.sbuf_tensor([128, 2 * 32], F32))
    sb_U = ctx.enter_context(nc.sbuf_tensor([128, 32], F32))
    sb_Vt = ctx.enter_context(nc.sbuf_tensor([128, 32], F32))
    sb_head = ctx.enter_context(nc.sbuf_tensor([11, T * BL], F32))

    ps_pre = [ctx.enter_context(nc.psum_tensor(f"ps_pre{j}", [128, 512], F32))
              for j in range(2)]
    ps_z = [ctx.enter_context(nc.psum_tensor(f"ps_z{j}", [128, 512], F32))
            for j in range(2)]
    ps_hd = [ctx.enter_context(nc.psum_tensor(f"ps_hd{j}", [128, 512], F32))
             for j in range(2)]

    # one semaphore per input load: DMA completion order is not program
    # order, so a shared counter would not identify which load finished.
    dma_xt = ctx.enter_context(nc.semaphore("dma_xt"))
    dma_wk = ctx.enter_context(nc.semaphore("dma_wk"))
    dma_wr = ctx.enter_context(nc.semaphore("dma_wr"))
    dma_bl = ctx.enter_context(nc.semaphore("dma_bl"))
    dma_wo = ctx.enter_context(nc.semaphore("dma_wo"))
    dma_bo = ctx.enter_context(nc.semaphore("dma_bo"))
    dma_out = ctx.enter_context(nc.semaphore("dma_out"))
    sem_pre = ctx.enter_context(nc.semaphore("sem_pre"))
    sem_precp = ctx.enter_context(nc.semaphore("sem_precp"))
    sem_h = ctx.enter_context(nc.semaphore("sem_h"))
    sem_zadd = ctx.enter_context(nc.semaphore("sem_zadd"))
    sem_cp = ctx.enter_context(nc.semaphore("sem_cp"))
    sem_act = ctx.enter_context(nc.semaphore("sem_act"))
    sem_pe = ctx.enter_context(nc.semaphore("sem_pe"))
    sem_hd = ctx.enter_context(nc.semaphore("sem_hd"))
    sem_hdcp = ctx.enter_context(nc.semaphore("sem_hdcp"))

    # number of 32-t halves in the head / precompute split
    n_half = 2
    assert T % n_half == 0
    TH = T // n_half  # t-steps per half (32 for T=64)
    assert TH * BL == HALF

    with nc.Block() as block:

        @block.sync
        def _(sync):
            sync.dma_start(out=sb_Xt[:], in_=d_Xt[:]).then_inc(dma_xt, 16)
            sync.dma_start(out=sb_WkT[:], in_=d_WkT[:]).then_inc(dma_wk, 16)
            sync.dma_start(out=sb_WrT[:], in_=d_WrT[:]).then_inc(dma_wr, 16)
            sync.dma_start(out=sb_blstm[:], in_=d_blstm[:]).then_inc(dma_bl, 16)
            sync.dma_start(out=sb_WoC[:], in_=d_WoC[:]).then_inc(dma_wo, 16)
            sync.dma_start(out=sb_bob[:], in_=d_bob[:]).then_inc(dma_bo, 16)
            sync.wait_ge(sem_hdcp, n_half)
            sync.dma_start(out=d_head[:], in_=sb_head[:]).then_inc(dma_out, 16)
            sync.wait_ge(sem_h, T + 1)
            sync.dma_start(out=d_state[:, 0:32],
                           in_=sb_H[:, T * 32:(T + 1) * 32]).then_inc(dma_out, 16)
            sync.wait_ge(sem_cp, T)
            sync.dma_start(out=d_state[:, 32:64],
                           in_=sb_C[:, T * 32:(T + 1) * 32]).then_inc(dma_out, 16)
            sync.wait_ge(dma_out, 48)

        @block.tensor
        def _(tensor):
            # ---- precompute XW = Wk^T X^T (feature-major) ----
            tensor.wait_ge(dma_xt, 16)
            tensor.wait_ge(dma_wk, 16)
            idx = 0
            for half in range(n_half):
                for m in range(M_TILES):
                    if idx >= 2:
                        tensor.wait_ge(sem_precp, idx - 1)
                    tensor.matmul(
                        ps_pre[idx % 2][:, 0:HALF],
                        sb_WkT[:, m * 128:(m + 1) * 128],
                        sb_Xt[:, half * HALF:(half + 1) * HALF],
                        start=True, stop=True,
                    ).then_inc(sem_pre)
                    idx += 1
            # ---- recurrent scan ----
            tensor.wait_ge(dma_wr, 16)
            for t in range(T):
                tensor.wait_ge(sem_h, t + 1)
                if t >= 2:
                    tensor.wait_ge(sem_zadd, t - 1)
                for m in range(M_TILES):
                    for k in range(K2):
                        mm = tensor.matmul(
                            ps_z[t % 2][:, m * BL:(m + 1) * BL],
                            sb_WrT[:, k * 1024 + m * 128:k * 1024 + (m + 1) * 128],
                            sb_H[:, t * 32 + k * BL:t * 32 + (k + 1) * BL],
                            start=(k == 0), stop=(k == 1),
                        )
                mm.then_inc(sem_pe)
            # ---- head: [Wo | Wc]^T applied to all h_t ----
            tensor.wait_ge(dma_wo, 16)
            tensor.wait_ge(sem_h, T + 1)
            h4 = sb_H[:].rearrange("p (t k b) -> p t k b", k=K2, b=BL)
            for half in range(n_half):
                for k in range(K2):
                    tensor.matmul(
                        ps_hd[half][0:11, 0:HALF],
                        sb_WoC[:, k * 11:(k + 1) * 11],
                        h4[:, 1 + half * TH:1 + (half + 1) * TH, k, :],
                        start=(k == 0), stop=(k == 1),
                    ).then_inc(sem_hd)

        @block.vector
        def _(vector):
            vector.memset(sb_H[:, 0:32], 0.0)
            vector.memset(sb_C[:, 0:32], 0.0).then_inc(sem_h)
            # ---- precompute copies: psum + b_lstm -> XW ----
            vector.wait_ge(dma_bl, 16)
            xw4 = sb_XW[:].rearrange("p (t m b) -> p t m b", m=M_TILES, b=BL)
            idx = 0
            for half in range(n_half):
                for m in range(M_TILES):
                    vector.wait_ge(sem_pre, idx + 1)
                    src = ps_pre[idx % 2][:, 0:HALF].rearrange(
                        "p (t b) -> p t b", b=BL)
                    nc.vector.tensor_scalar_add(
                        xw4[:, half * TH:(half + 1) * TH, m, :], src,
                        sb_blstm[:, m:m + 1],
                    ).then_inc(sem_precp)
                    idx += 1
            # same-engine RAW fence: scan reads XW / C(memset) written above
            vector.drain()
            # ---- scan pointwise ----
            for t in range(T):
                s = t % 2
                zs = sb_Z[:, s * 128:(s + 1) * 128]
                gs = sb_G[:, s * 128:(s + 1) * 128]
                vector.wait_ge(sem_pe, t + 1)
                nc.vector.tensor_add(zs, ps_z[s][:, 0:128],
                                     sb_XW[:, t * 128:(t + 1) * 128]
                                     ).then_inc(sem_zadd)
                # fence: v reads C written by previous step's c'-add
                vector.drain()
                vector.wait_ge(sem_act, 3 * t + 1)
                nc.vector.tensor_mul(sb_Vt[:], gs[:, 32:64],
                                     sb_C[:, t * 32:(t + 1) * 32])
                vector.wait_ge(sem_act, 3 * t + 2)
                nc.vector.tensor_mul(sb_U[:], gs[:, 0:32], gs[:, 96:128])
                # fence: c'-add reads U/V written just above
                vector.drain()
                nc.vector.tensor_add(sb_C[:, (t + 1) * 32:(t + 2) * 32],
                                     sb_U[:], sb_Vt[:]).then_inc(sem_cp)
                vector.wait_ge(sem_act, 3 * t + 3)
                nc.vector.tensor_mul(sb_H[:, (t + 1) * 32:(t + 2) * 32],
                                     gs[:, 64:96],
                                     sb_TC[:, s * 32:(s + 1) * 32]
                                     ).then_inc(sem_h)
            # ---- head copies ----
            vector.wait_ge(dma_bo, 16)
            for half in range(n_half):
                vector.wait_ge(sem_hd, K2 * (half + 1))
                nc.vector.tensor_scalar_add(
                    sb_head[:, half * HALF:(half + 1) * HALF],
                    ps_hd[half][0:11, 0:HALF], sb_bob[0:11, 0:1],
                ).then_inc(sem_hdcp)

        @block.scalar
        def _(scalar):
            Sig = mybir.ActivationFunctionType.Sigmoid
            Tanh = mybir.ActivationFunctionType.Tanh
            for t in range(T):
                s = t % 2
                zs = sb_Z[:, s * 128:(s + 1) * 128]
                gs = sb_G[:, s * 128:(s + 1) * 128]
                scalar.wait_ge(sem_zadd, t + 1)
                scalar.activation(gs[:, 0:96], zs[:, 0:96], Sig).then_inc(sem_act)
                scalar.activation(gs[:, 96:128], zs[:, 96:128], Tanh
                                  ).then_inc(sem_act)
                scalar.wait_ge(sem_cp, t + 1)
                scalar.activation(sb_TC[:, s * 32:(s + 1) * 32],
                                  sb_C[:, (t + 1) * 32:(t + 2) * 32], Tanh
                                  ).then_inc(sem_act)

    return nc, ctx


_BUILD_CACHE = {}


def _get_nc(T):
    if T not in _BUILD_CACHE:
        _BUILD_CACHE[T] = _build(T)
    return _BUILD_CACHE[T][0]


def _prep_inputs(X, u, Wk, Wr, b_lstm, Wo, bo, Wc, bc, T):
    """Build the 8 per-core input maps (numpy, host-side sharding)."""
    Wk_p = np.ascontiguousarray(Wk[:, GATE_PERM], np.float32)
    Wr_p = Wr[:, GATE_PERM].astype(np.float32)
    WrT = np.ascontiguousarray(
        Wr_p.reshape(2, 128, 1024).transpose(1, 0, 2).reshape(128, 2048))
    blstm = np.ascontiguousarray(
        b_lstm[GATE_PERM].astype(np.float32).reshape(8, 128).T)
    WoC = np.concatenate([Wo.astype(np.float32),
                          Wc[:256].astype(np.float32)], axis=1)  # [256, 11]
    WoC = np.ascontiguousarray(
        WoC.reshape(2, 128, 11).transpose(1, 0, 2).reshape(128, 22))
    bob = np.concatenate([bo.astype(np.float32), [0.0]]).reshape(11, 1)
    bob = np.ascontiguousarray(bob, np.float32)

    in_maps = []
    for i in range(NCORES):
        bsl = slice(i * BL, (i + 1) * BL)
        Xt = np.ascontiguousarray(
            X[bsl, :T, :].astype(np.float32).transpose(2, 1, 0).reshape(128, T * BL))
        in_maps.append({
            "Xt": Xt, "WkT": Wk_p, "WrT": WrT, "blstm": blstm,
            "WoC": WoC, "bob": bob,
        })
    return in_maps


def _sigmoid64(x):
    return 1.0 / (1.0 + np.exp(-x.astype(np.float64)))


def _softmax32(x):
    x = x.astype(np.float32)
    e = np.exp(x - x.max(axis=-1, keepdims=True))
    return (e / e.sum(axis=-1, keepdims=True)).astype(np.float32)


def _fallback_scan(x_seq, u_seq, h0, c0, t0, Wk, Wr, b_lstm, Wo, bo, Wc, bc):
    """Continue the reference recurrence on host for one sample that did not
    halt by t0.  Returns the sample's output row (float32)."""
    h = h0.astype(np.float32).copy()
    c = c0.astype(np.float32).copy()
    Wk = Wk.astype(np.float32); Wr = Wr.astype(np.float32)
    b_lstm = b_lstm.astype(np.float32)
    sig = lambda v: 1.0 / (1.0 + np.exp(-v))
    Tt = x_seq.shape[0]
    logits_last = None
    for t in range(t0, Tt):
        z = x_seq[t] @ Wk + h @ Wr + b_lstm
        i, f, g, o = np.split(z, 4)
        i = sig(i); f = sig(f); g = np.tanh(g); o = sig(o)
        c = f * c + i * g
        h = o * np.tanh(c)
        y = h @ Wo.astype(np.float32) + bo.astype(np.float32)
        logits = _softmax32(y)
        pre = float(h @ Wc[:256, 0].astype(np.float32)) \
            + t * float(Wc[256, 0]) + float(bc[0])
        probs = (1.0 - EPS) * sig(np.float32(pre)) + EPS * 0.05
        if u_seq[t] < probs:
            return logits
        logits_last = logits
    return logits_last


def kernel(**inputs):
    X = np.asarray(inputs["X"], np.float32)
    u = np.asarray(inputs["u"], np.float32)
    Wk = np.asarray(inputs["Wk"], np.float32)
    Wr = np.asarray(inputs["Wr"], np.float32)
    b_lstm = np.asarray(inputs["b_lstm"], np.float32)
    Wo = np.asarray(inputs["Wo"], np.float32)
    bo = np.asarray(inputs["bo"], np.float32)
    Wc = np.asarray(inputs["Wc"], np.float32)
    bc = np.asarray(inputs["bc"], np.float32)
    T = T_EFF

    nc = _get_nc(T)
    in_maps = _prep_inputs(X, u, Wk, Wr, b_lstm, Wo, bo, Wc, bc, T)
    res = run_bass_kernel_spmd(nc, in_maps, list(range(NCORES)))

    wc_t = float(Wc[256, 0])
    bias_c = float(bc[0])
    tvec = np.arange(T, dtype=np.float64)

    out = np.zeros((B, C), np.float32)
    for i in range(NCORES):
        bsl = slice(i * BL, (i + 1) * BL)
        head = res.results[i]["head"]          # [11, T*BL]
        y_pre = head[0:10].reshape(10, T, BL).transpose(1, 2, 0)  # [T, b, 10]
        pre_c = head[10].reshape(T, BL).astype(np.float64)        # [T, b]
        probs = (1.0 - EPS) * _sigmoid64(pre_c + tvec[:, None] * wc_t + bias_c) \
            + EPS * 0.05
        u_core = u[bsl, :T, 0]                 # [b, T]
        a = u_core.T.astype(np.float64) < probs  # [T, b]
        halted = a.any(axis=0)
        tstar = np.argmax(a, axis=0)           # first halt step per sample
        logits = _softmax32(y_pre)             # [T, b, 10]
        for b_ in range(BL):
            if halted[b_]:
                out[i * BL + b_] = logits[tstar[b_], b_]
            else:
                state = res.results[i]["state"]  # [128, 64]
                h_T = state[:, 0:32].reshape(128, 2, BL).transpose(2, 1, 0) \
                    .reshape(BL, 256)[b_]
                c_T = state[:, 32:64].reshape(128, 2, BL).transpose(2, 1, 0) \
                    .reshape(BL, 256)[b_]
                out[i * BL + b_] = _fallback_scan(
                    X[i * BL + b_], u[i * BL + b_, :, 0], h_T, c_T, T,
                    Wk, Wr, b_lstm, Wo, bo, Wc, bc)
    return out


# revision 8
# speedup vs baseline: 2.6238x; 2.6238x over previous
# Trainium2 Bass kernel for nn_EARLIEST (adaptive-halting LSTM, B=128 T=4096
# V=128 H=256 C=10).
#
# Key observation: the model halts each batch sample at the first step t where
# u[b,t] < probs[b,t], with probs ~= 0.45 early on, so every sample halts
# within a few dozen steps (max 36 for the seed-0 inputs).  The returned
# output only needs logits at each sample's first halt step (or step T-1 for
# never-halted samples).  So the device kernel runs the LSTM scan for only
# T_EFF timesteps, emits pre-softmax logits and the halting dot-product for
# every (t, b), and the host applies the (exact) halting latch.  A numpy
# fallback continues the recurrence from the device's (h, c) state for any
# sample that has not halted by T_EFF (statistically never happens; the
# fallback keeps the kernel correct for arbitrary inputs).
#
# Sharding: data-parallel over batch, 16 samples per core, weights replicated.
# Layout on device is feature-major: h^T is [H=256, b=16] stored as two
# 128-partition k-tiles side by side, so LSTM gate math runs on full
# 128-partition tiles and the recurrent matmuls need no transposes.

import numpy as np
import ml_dtypes

import concourse.bass as bass
import concourse.mybir as mybir
from concourse.bass_utils import run_bass_kernel_spmd

B, T_FULL, V, H, C = 128, 4096, 128, 256, 10
EPS = 0.1
NCORES = 8
BL = B // NCORES  # 16 samples per core
T_EFF = 64
M_TILES = 8   # 4H/128
K2 = 2        # H/128
F32 = mybir.dt.float32
BF16 = mybir.dt.bfloat16

# gate order in z is (i, f, g, o); we permute weight columns to (i, f, o, g)
# so one sigmoid covers a contiguous [i|f|o] block and one tanh covers g.
GATE_PERM = np.concatenate([np.arange(0, 512), np.arange(768, 1024),
                            np.arange(512, 768)])


def _build(T):
    """Build the raw-bass single-core program (SPMD across 8 cores)."""
    nc = bass.Bass()

    d_Xt = nc.dram_tensor("Xt", [128, T * BL], F32, kind="ExternalInput")
    d_WkT = nc.dram_tensor("WkT", [128, 1024], F32, kind="ExternalInput")
    d_WrT = nc.dram_tensor("WrT", [128, 2048], BF16, kind="ExternalInput")
    d_blstm = nc.dram_tensor("blstm", [128, 8], F32, kind="ExternalInput")
    d_WoC = nc.dram_tensor("WoC", [128, 22], BF16, kind="ExternalInput")
    d_bob = nc.dram_tensor("bob", [11, 1], F32, kind="ExternalInput")
    d_head = nc.dram_tensor("head", [11, T * BL], F32, kind="ExternalOutput")
    d_state_h = nc.dram_tensor("state_h", [128, 32], BF16, kind="ExternalOutput")
    d_state_c = nc.dram_tensor("state_c", [128, 32], F32, kind="ExternalOutput")

    NH = T * BL  # head columns
    HALF = NH // 2  # 512 per fp32-matmul moving limit

    from contextlib import ExitStack
    ctx = ExitStack()
    sb_Xt = ctx.enter_context(nc.sbuf_tensor([128, T * BL], F32))
    sb_WkT = ctx.enter_context(nc.sbuf_tensor([128, 1024], F32))
    sb_WrT = ctx.enter_context(nc.sbuf_tensor([128, 2048], BF16))
    sb_blstm = ctx.enter_context(nc.sbuf_tensor([128, 8], F32))
    sb_WoC = ctx.enter_context(nc.sbuf_tensor([128, 22], BF16))
    sb_bob = ctx.enter_context(nc.sbuf_tensor([11, 1], F32))
    sb_XW = ctx.enter_context(nc.sbuf_tensor([128, T * 128], F32))
    sb_H = ctx.enter_context(nc.sbuf_tensor([128, (T + 1) * 32], BF16))
    sb_C = ctx.enter_context(nc.sbuf_tensor([128, (T + 1) * 32], F32))
    sb_Z = ctx.enter_context(nc.sbuf_tensor([128, 2 * 128], F32))
    sb_G = ctx.enter_context(nc.sbuf_tensor([128, 2 * 128], F32))
    sb_TC = ctx.enter_context(nc.sbuf_tensor([128, 2 * 32], F32))
    sb_U = ctx.enter_context(nc.sbuf_tensor([128, 32], F32))
    sb_Vt = ctx.enter_context(nc.sbuf_tensor([128, 32], F32))
    sb_head = ctx.enter_context(nc.sbuf_tensor([11, T * BL], F32))

    ps_pre = [ctx.enter_context(nc.psum_tensor(f"ps_pre{j}", [128, 512], F32))
              for j in range(2)]
    ps_z = [ctx.enter_context(nc.psum_tensor(f"ps_z{j}", [128, 512], F32))
            for j in range(2)]
    ps_hd = [ctx.enter_context(nc.psum_tensor(f"ps_hd{j}", [128, 512], F32))
             for j in range(2)]

    # one semaphore per input load: DMA completion order is not program
    # order, so a shared counter would not identify which load finished.
    dma_xt = ctx.enter_context(nc.semaphore("dma_xt"))
    dma_wk = ctx.enter_context(nc.semaphore("dma_wk"))
    dma_wr = ctx.enter_context(nc.semaphore("dma_wr"))
    dma_bl = ctx.enter_context(nc.semaphore("dma_bl"))
    dma_wo = ctx.enter_context(nc.semaphore("dma_wo"))
    dma_bo = ctx.enter_context(nc.semaphore("dma_bo"))
    dma_out = ctx.enter_context(nc.semaphore("dma_out"))
    sem_pre = ctx.enter_context(nc.semaphore("sem_pre"))
    sem_precp = ctx.enter_context(nc.semaphore("sem_precp"))
    sem_h = ctx.enter_context(nc.semaphore("sem_h"))
    sem_zadd = ctx.enter_context(nc.semaphore("sem_zadd"))
    sem_cp = ctx.enter_context(nc.semaphore("sem_cp"))
    sem_act = ctx.enter_context(nc.semaphore("sem_act"))
    sem_pe = ctx.enter_context(nc.semaphore("sem_pe"))
    sem_hd = ctx.enter_context(nc.semaphore("sem_hd"))
    sem_hdcp = ctx.enter_context(nc.semaphore("sem_hdcp"))

    # number of 32-t halves in the head / precompute split
    n_half = 2
    assert T % n_half == 0
    TH = T // n_half  # t-steps per half (32 for T=64)
    assert TH * BL == HALF

    with nc.Block() as block:

        @block.sync
        def _(sync):
            sync.dma_start(out=sb_Xt[:], in_=d_Xt[:]).then_inc(dma_xt, 16)
            sync.dma_start(out=sb_WkT[:], in_=d_WkT[:]).then_inc(dma_wk, 16)
            sync.dma_start(out=sb_WrT[:], in_=d_WrT[:]).then_inc(dma_wr, 16)
            sync.dma_start(out=sb_blstm[:], in_=d_blstm[:]).then_inc(dma_bl, 16)
            sync.dma_start(out=sb_WoC[:], in_=d_WoC[:]).then_inc(dma_wo, 16)
            sync.dma_start(out=sb_bob[:], in_=d_bob[:]).then_inc(dma_bo, 16)
            sync.wait_ge(sem_hdcp, n_half)
            sync.dma_start(out=d_head[:], in_=sb_head[:]).then_inc(dma_out, 16)
            sync.wait_ge(sem_h, T + 1)
            sync.dma_start(out=d_state_h[:],
                           in_=sb_H[:, T * 32:(T + 1) * 32]).then_inc(dma_out, 16)
            sync.wait_ge(sem_cp, T)
            sync.dma_start(out=d_state_c[:],
                           in_=sb_C[:, T * 32:(T + 1) * 32]).then_inc(dma_out, 16)
            sync.wait_ge(dma_out, 48)

        @block.tensor
        def _(tensor):
            # ---- precompute XW = Wk^T X^T (feature-major) ----
            tensor.wait_ge(dma_xt, 16)
            tensor.wait_ge(dma_wk, 16)
            idx = 0
            for half in range(n_half):
                for m in range(M_TILES):
                    if idx >= 2:
                        tensor.wait_ge(sem_precp, idx - 1)
                    tensor.matmul(
                        ps_pre[idx % 2][:, 0:HALF],
                        sb_WkT[:, m * 128:(m + 1) * 128],
                        sb_Xt[:, half * HALF:(half + 1) * HALF],
                        start=True, stop=True,
                    ).then_inc(sem_pre)
                    idx += 1
            # ---- recurrent scan ----
            tensor.wait_ge(dma_wr, 16)
            for t in range(T):
                tensor.wait_ge(sem_h, t + 1)
                if t >= 2:
                    tensor.wait_ge(sem_zadd, t - 1)
                for m in range(M_TILES):
                    for k in range(K2):
                        mm = tensor.matmul(
                            ps_z[t % 2][:, m * BL:(m + 1) * BL],
                            sb_WrT[:, k * 1024 + m * 128:k * 1024 + (m + 1) * 128],
                            sb_H[:, t * 32 + k * BL:t * 32 + (k + 1) * BL],
                            start=(k == 0), stop=(k == 1),
                        )
                mm.then_inc(sem_pe)
            # ---- head: [Wo | Wc]^T applied to all h_t ----
            tensor.wait_ge(dma_wo, 16)
            tensor.wait_ge(sem_h, T + 1)
            h4 = sb_H[:].rearrange("p (t k b) -> p t k b", k=K2, b=BL)
            for half in range(n_half):
                for k in range(K2):
                    tensor.matmul(
                        ps_hd[half][0:11, 0:HALF],
                        sb_WoC[:, k * 11:(k + 1) * 11],
                        h4[:, 1 + half * TH:1 + (half + 1) * TH, k, :],
                        start=(k == 0), stop=(k == 1),
                    ).then_inc(sem_hd)

        @block.vector
        def _(vector):
            vector.memset(sb_H[:, 0:32], 0.0)
            vector.memset(sb_C[:, 0:32], 0.0).then_inc(sem_h)
            # ---- precompute copies: psum + b_lstm -> XW ----
            vector.wait_ge(dma_bl, 16)
            xw4 = sb_XW[:].rearrange("p (t m b) -> p t m b", m=M_TILES, b=BL)
            idx = 0
            for half in range(n_half):
                for m in range(M_TILES):
                    vector.wait_ge(sem_pre, idx + 1)
                    src = ps_pre[idx % 2][:, 0:HALF].rearrange(
                        "p (t b) -> p t b", b=BL)
                    nc.vector.tensor_scalar_add(
                        xw4[:, half * TH:(half + 1) * TH, m, :], src,
                        sb_blstm[:, m:m + 1],
                    ).then_inc(sem_precp)
                    idx += 1
            # same-engine RAW fence: scan reads XW / C(memset) written above
            vector.drain()
            # ---- scan pointwise ----
            for t in range(T):
                s = t % 2
                zs = sb_Z[:, s * 128:(s + 1) * 128]
                gs = sb_G[:, s * 128:(s + 1) * 128]
                vector.wait_ge(sem_pe, t + 1)
                nc.vector.tensor_add(zs, ps_z[s][:, 0:128],
                                     sb_XW[:, t * 128:(t + 1) * 128]
                                     ).then_inc(sem_zadd)
                # fence: v reads C written by previous step's c'-add
                vector.drain()
                vector.wait_ge(sem_act, 3 * t + 1)
                nc.vector.tensor_mul(sb_Vt[:], gs[:, 32:64],
                                     sb_C[:, t * 32:(t + 1) * 32])
                vector.wait_ge(sem_act, 3 * t + 2)
                nc.vector.tensor_mul(sb_U[:], gs[:, 0:32], gs[:, 96:128])
                # fence: c'-add reads U/V written just above
                vector.drain()
                nc.vector.tensor_add(sb_C[:, (t + 1) * 32:(t + 2) * 32],
                                     sb_U[:], sb_Vt[:]).then_inc(sem_cp)
                vector.wait_ge(sem_act, 3 * t + 3)
                nc.vector.tensor_mul(sb_H[:, (t + 1) * 32:(t + 2) * 32],
                                     gs[:, 64:96],
                                     sb_TC[:, s * 32:(s + 1) * 32]
                                     ).then_inc(sem_h)
            # ---- head copies ----
            vector.wait_ge(dma_bo, 16)
            for half in range(n_half):
                vector.wait_ge(sem_hd, K2 * (half + 1))
                nc.vector.tensor_scalar_add(
                    sb_head[:, half * HALF:(half + 1) * HALF],
                    ps_hd[half][0:11, 0:HALF], sb_bob[0:11, 0:1],
                ).then_inc(sem_hdcp)

        @block.scalar
        def _(scalar):
            Sig = mybir.ActivationFunctionType.Sigmoid
            Tanh = mybir.ActivationFunctionType.Tanh
            for t in range(T):
                s = t % 2
                zs = sb_Z[:, s * 128:(s + 1) * 128]
                gs = sb_G[:, s * 128:(s + 1) * 128]
                scalar.wait_ge(sem_zadd, t + 1)
                scalar.activation(gs[:, 0:96], zs[:, 0:96], Sig).then_inc(sem_act)
                scalar.activation(gs[:, 96:128], zs[:, 96:128], Tanh
                                  ).then_inc(sem_act)
                scalar.wait_ge(sem_cp, t + 1)
                scalar.activation(sb_TC[:, s * 32:(s + 1) * 32],
                                  sb_C[:, (t + 1) * 32:(t + 2) * 32], Tanh
                                  ).then_inc(sem_act)

    return nc, ctx


_BUILD_CACHE = {}


def _get_nc(T):
    if T not in _BUILD_CACHE:
        _BUILD_CACHE[T] = _build(T)
    return _BUILD_CACHE[T][0]


def _prep_inputs(X, u, Wk, Wr, b_lstm, Wo, bo, Wc, bc, T):
    """Build the 8 per-core input maps (numpy, host-side sharding)."""
    Wk_p = np.ascontiguousarray(Wk[:, GATE_PERM], np.float32)
    Wr_p = Wr[:, GATE_PERM].astype(np.float32)
    WrT = np.ascontiguousarray(
        Wr_p.reshape(2, 128, 1024).transpose(1, 0, 2).reshape(128, 2048)
    ).astype(ml_dtypes.bfloat16)
    blstm = np.ascontiguousarray(
        b_lstm[GATE_PERM].astype(np.float32).reshape(8, 128).T)
    WoC = np.concatenate([Wo.astype(np.float32),
                          Wc[:256].astype(np.float32)], axis=1)  # [256, 11]
    WoC = np.ascontiguousarray(
        WoC.reshape(2, 128, 11).transpose(1, 0, 2).reshape(128, 22)
    ).astype(ml_dtypes.bfloat16)
    bob = np.concatenate([bo.astype(np.float32), [0.0]]).reshape(11, 1)
    bob = np.ascontiguousarray(bob, np.float32)

    in_maps = []
    for i in range(NCORES):
        bsl = slice(i * BL, (i + 1) * BL)
        Xt = np.ascontiguousarray(
            X[bsl, :T, :].astype(np.float32).transpose(2, 1, 0).reshape(128, T * BL))
        in_maps.append({
            "Xt": Xt, "WkT": Wk_p, "WrT": WrT, "blstm": blstm,
            "WoC": WoC, "bob": bob,
        })
    return in_maps


def _sigmoid64(x):
    return 1.0 / (1.0 + np.exp(-x.astype(np.float64)))


def _softmax32(x):
    x = x.astype(np.float32)
    e = np.exp(x - x.max(axis=-1, keepdims=True))
    return (e / e.sum(axis=-1, keepdims=True)).astype(np.float32)


def _fallback_scan(x_seq, u_seq, h0, c0, t0, Wk, Wr, b_lstm, Wo, bo, Wc, bc):
    """Continue the reference recurrence on host for one sample that did not
    halt by t0.  Returns the sample's output row (float32)."""
    h = h0.astype(np.float32).copy()
    c = c0.astype(np.float32).copy()
    Wk = Wk.astype(np.float32); Wr = Wr.astype(np.float32)
    b_lstm = b_lstm.astype(np.float32)
    sig = lambda v: 1.0 / (1.0 + np.exp(-v))
    Tt = x_seq.shape[0]
    logits_last = None
    for t in range(t0, Tt):
        z = x_seq[t] @ Wk + h @ Wr + b_lstm
        i, f, g, o = np.split(z, 4)
        i = sig(i); f = sig(f); g = np.tanh(g); o = sig(o)
        c = f * c + i * g
        h = o * np.tanh(c)
        y = h @ Wo.astype(np.float32) + bo.astype(np.float32)
        logits = _softmax32(y)
        pre = float(h @ Wc[:256, 0].astype(np.float32)) \
            + t * float(Wc[256, 0]) + float(bc[0])
        probs = (1.0 - EPS) * sig(np.float32(pre)) + EPS * 0.05
        if u_seq[t] < probs:
            return logits
        logits_last = logits
    return logits_last


def kernel(**inputs):
    X = np.asarray(inputs["X"], np.float32)
    u = np.asarray(inputs["u"], np.float32)
    Wk = np.asarray(inputs["Wk"], np.float32)
    Wr = np.asarray(inputs["Wr"], np.float32)
    b_lstm = np.asarray(inputs["b_lstm"], np.float32)
    Wo = np.asarray(inputs["Wo"], np.float32)
    bo = np.asarray(inputs["bo"], np.float32)
    Wc = np.asarray(inputs["Wc"], np.float32)
    bc = np.asarray(inputs["bc"], np.float32)
    T = T_EFF

    nc = _get_nc(T)
    in_maps = _prep_inputs(X, u, Wk, Wr, b_lstm, Wo, bo, Wc, bc, T)
    res = run_bass_kernel_spmd(nc, in_maps, list(range(NCORES)))

    wc_t = float(Wc[256, 0])
    bias_c = float(bc[0])
    tvec = np.arange(T, dtype=np.float64)

    out = np.zeros((B, C), np.float32)
    for i in range(NCORES):
        bsl = slice(i * BL, (i + 1) * BL)
        head = res.results[i]["head"]          # [11, T*BL]
        y_pre = head[0:10].reshape(10, T, BL).transpose(1, 2, 0)  # [T, b, 10]
        pre_c = head[10].reshape(T, BL).astype(np.float64)        # [T, b]
        probs = (1.0 - EPS) * _sigmoid64(pre_c + tvec[:, None] * wc_t + bias_c) \
            + EPS * 0.05
        u_core = u[bsl, :T, 0]                 # [b, T]
        a = u_core.T.astype(np.float64) < probs  # [T, b]
        halted = a.any(axis=0)
        tstar = np.argmax(a, axis=0)           # first halt step per sample
        logits = _softmax32(y_pre)             # [T, b, 10]
        for b_ in range(BL):
            if halted[b_]:
                out[i * BL + b_] = logits[tstar[b_], b_]
            else:
                sh = res.results[i]["state_h"].astype(np.float32)
                sc = res.results[i]["state_c"].astype(np.float32)
                h_T = sh.reshape(128, 2, BL).transpose(2, 1, 0) \
                    .reshape(BL, 256)[b_]
                c_T = sc.reshape(128, 2, BL).transpose(2, 1, 0) \
                    .reshape(BL, 256)[b_]
                out[i * BL + b_] = _fallback_scan(
                    X[i * BL + b_], u[i * BL + b_, :, 0], h_T, c_T, T,
                    Wk, Wr, b_lstm, Wo, bo, Wc, bc)
    return out


# revision 9
# speedup vs baseline: 2.6486x; 1.0094x over previous
# Trainium2 Bass kernel for nn_EARLIEST (adaptive-halting LSTM, B=128 T=4096
# V=128 H=256 C=10).
#
# Key observation: the model halts each batch sample at the first step t where
# u[b,t] < probs[b,t], with probs ~= 0.45 early on, so every sample halts
# within a few dozen steps (max 36 for the seed-0 inputs).  The returned
# output only needs logits at each sample's first halt step (or step T-1 for
# never-halted samples).  So the device kernel runs the LSTM scan for only
# T_EFF timesteps, emits pre-softmax logits and the halting dot-product for
# every (t, b), and the host applies the (exact) halting latch.  A numpy
# fallback continues the recurrence from the device's (h, c) state for any
# sample that has not halted by T_EFF (statistically never happens; the
# fallback keeps the kernel correct for arbitrary inputs).
#
# Sharding: data-parallel over batch, 16 samples per core, weights replicated.
# Layout on device is feature-major: h^T is [H=256, b=16] stored as two
# 128-partition k-tiles side by side, so LSTM gate math runs on full
# 128-partition tiles and the recurrent matmuls need no transposes.

import numpy as np
import ml_dtypes

import concourse.bass as bass
import concourse.mybir as mybir
from concourse.bass_utils import run_bass_kernel_spmd

B, T_FULL, V, H, C = 128, 4096, 128, 256, 10
EPS = 0.1
NCORES = 8
BL = B // NCORES  # 16 samples per core
T_EFF = 64
M_TILES = 8   # 4H/128
K2 = 2        # H/128
F32 = mybir.dt.float32
BF16 = mybir.dt.bfloat16

# gate order in z is (i, f, g, o); we permute weight columns to (i, f, o, g)
# so one sigmoid covers a contiguous [i|f|o] block and one tanh covers g.
GATE_PERM = np.concatenate([np.arange(0, 512), np.arange(768, 1024),
                            np.arange(512, 768)])


def _build(T):
    """Build the raw-bass single-core program (SPMD across 8 cores)."""
    nc = bass.Bass()

    d_Xt = nc.dram_tensor("Xt", [128, T * BL], F32, kind="ExternalInput")
    d_WkT = nc.dram_tensor("WkT", [128, 1024], F32, kind="ExternalInput")
    d_WrT = nc.dram_tensor("WrT", [128, 2048], BF16, kind="ExternalInput")
    d_blstm = nc.dram_tensor("blstm", [128, 8], F32, kind="ExternalInput")
    d_WoC = nc.dram_tensor("WoC", [128, 22], BF16, kind="ExternalInput")
    d_bob = nc.dram_tensor("bob", [11, 1], F32, kind="ExternalInput")
    d_head = nc.dram_tensor("head", [11, T * BL], F32, kind="ExternalOutput")
    d_state_h = nc.dram_tensor("state_h", [128, 32], BF16, kind="ExternalOutput")
    d_state_c = nc.dram_tensor("state_c", [128, 32], F32, kind="ExternalOutput")

    NH = T * BL  # head columns
    HALF = NH // 2  # 512 per fp32-matmul moving limit

    from contextlib import ExitStack
    ctx = ExitStack()
    sb_Xt = ctx.enter_context(nc.sbuf_tensor([128, T * BL], F32))
    sb_WkT = ctx.enter_context(nc.sbuf_tensor([128, 1024], F32))
    sb_WrT = ctx.enter_context(nc.sbuf_tensor([128, 2048], BF16))
    sb_blstm = ctx.enter_context(nc.sbuf_tensor([128, 8], F32))
    sb_WoC = ctx.enter_context(nc.sbuf_tensor([128, 22], BF16))
    sb_bob = ctx.enter_context(nc.sbuf_tensor([11, 1], F32))
    sb_XW = ctx.enter_context(nc.sbuf_tensor([128, T * 128], F32))
    sb_H = ctx.enter_context(nc.sbuf_tensor([128, (T + 1) * 32], BF16))
    sb_C = ctx.enter_context(nc.sbuf_tensor([128, (T + 1) * 32], F32))
    sb_Z = ctx.enter_context(nc.sbuf_tensor([128, 2 * 128], F32))
    sb_G = ctx.enter_context(nc.sbuf_tensor([128, 2 * 128], F32))
    sb_TC = ctx.enter_context(nc.sbuf_tensor([128, 2 * 32], F32))
    sb_U = ctx.enter_context(nc.sbuf_tensor([128, 32], F32))
    sb_Vt = ctx.enter_context(nc.sbuf_tensor([128, 32], F32))
    sb_head = ctx.enter_context(nc.sbuf_tensor([11, T * BL], F32))

    ps_pre = [ctx.enter_context(nc.psum_tensor(f"ps_pre{j}", [128, 512], F32))
              for j in range(2)]
    ps_z = [ctx.enter_context(nc.psum_tensor(f"ps_z{j}", [128, 512], F32))
            for j in range(2)]
    ps_hd = [ctx.enter_context(nc.psum_tensor(f"ps_hd{j}", [128, 512], F32))
             for j in range(2)]
    ps_zs = [ctx.enter_context(nc.psum_tensor(f"ps_zs{j}", [128, 512], F32))
             for j in range(2)]

    # one semaphore per input load: DMA completion order is not program
    # order, so a shared counter would not identify which load finished.
    dma_xt = ctx.enter_context(nc.semaphore("dma_xt"))
    dma_wk = ctx.enter_context(nc.semaphore("dma_wk"))
    dma_wr = ctx.enter_context(nc.semaphore("dma_wr"))
    dma_bl = ctx.enter_context(nc.semaphore("dma_bl"))
    dma_wo = ctx.enter_context(nc.semaphore("dma_wo"))
    dma_bo = ctx.enter_context(nc.semaphore("dma_bo"))
    dma_out = ctx.enter_context(nc.semaphore("dma_out"))
    sem_pre = ctx.enter_context(nc.semaphore("sem_pre"))
    sem_precp = ctx.enter_context(nc.semaphore("sem_precp"))
    sem_h = ctx.enter_context(nc.semaphore("sem_h"))
    sem_zadd = ctx.enter_context(nc.semaphore("sem_zadd"))
    sem_cp = ctx.enter_context(nc.semaphore("sem_cp"))
    sem_act = ctx.enter_context(nc.semaphore("sem_act"))
    sem_pe = ctx.enter_context(nc.semaphore("sem_pe"))
    sem_hd = ctx.enter_context(nc.semaphore("sem_hd"))
    sem_hdcp = ctx.enter_context(nc.semaphore("sem_hdcp"))
    sem_uv = ctx.enter_context(nc.semaphore("sem_uv"))

    # number of 32-t halves in the head / precompute split
    n_half = 2
    assert T % n_half == 0
    TH = T // n_half  # t-steps per half (32 for T=64)
    assert TH * BL == HALF

    with nc.Block() as block:

        @block.sync
        def _(sync):
            sync.dma_start(out=sb_Xt[:], in_=d_Xt[:]).then_inc(dma_xt, 16)
            sync.dma_start(out=sb_WkT[:], in_=d_WkT[:]).then_inc(dma_wk, 16)
            sync.dma_start(out=sb_WrT[:], in_=d_WrT[:]).then_inc(dma_wr, 16)
            sync.dma_start(out=sb_blstm[:], in_=d_blstm[:]).then_inc(dma_bl, 16)
            sync.dma_start(out=sb_WoC[:], in_=d_WoC[:]).then_inc(dma_wo, 16)
            sync.dma_start(out=sb_bob[:], in_=d_bob[:]).then_inc(dma_bo, 16)
            sync.wait_ge(sem_hdcp, n_half)
            sync.dma_start(out=d_head[:], in_=sb_head[:]).then_inc(dma_out, 16)
            sync.wait_ge(sem_h, T + 1)
            sync.dma_start(out=d_state_h[:],
                           in_=sb_H[:, T * 32:(T + 1) * 32]).then_inc(dma_out, 16)
            sync.wait_ge(sem_cp, T)
            sync.dma_start(out=d_state_c[:],
                           in_=sb_C[:, T * 32:(T + 1) * 32]).then_inc(dma_out, 16)
            sync.wait_ge(dma_out, 48)

        @block.tensor
        def _(tensor):
            # ---- precompute XW = Wk^T X^T (feature-major) ----
            tensor.wait_ge(dma_xt, 16)
            tensor.wait_ge(dma_wk, 16)
            idx = 0
            for half in range(n_half):
                for m in range(M_TILES):
                    if idx >= 2:
                        tensor.wait_ge(sem_precp, idx - 1)
                    tensor.matmul(
                        ps_pre[idx % 2][:, 0:HALF],
                        sb_WkT[:, m * 128:(m + 1) * 128],
                        sb_Xt[:, half * HALF:(half + 1) * HALF],
                        start=True, stop=True,
                    ).then_inc(sem_pre)
                    idx += 1
            # ---- recurrent scan ----
            tensor.wait_ge(dma_wr, 16)
            for t in range(T):
                tensor.wait_ge(sem_h, t + 1)
                if t >= 2:
                    tensor.wait_ge(sem_zadd, t - 1)
                for m in range(M_TILES):
                    for k in range(K2):
                        mm = tensor.matmul(
                            ps_z[t % 2][:, m * BL:(m + 1) * BL],
                            sb_WrT[:, k * 1024 + m * 128:k * 1024 + (m + 1) * 128],
                            sb_H[:, t * 32 + k * BL:t * 32 + (k + 1) * BL],
                            start=(k == 0), stop=(k == 1),
                        )
                mm.then_inc(sem_pe)
            # ---- head: [Wo | Wc]^T applied to all h_t ----
            tensor.wait_ge(dma_wo, 16)
            tensor.wait_ge(sem_h, T + 1)
            h4 = sb_H[:].rearrange("p (t k b) -> p t k b", k=K2, b=BL)
            for half in range(n_half):
                for k in range(K2):
                    tensor.matmul(
                        ps_hd[half][0:11, 0:HALF],
                        sb_WoC[:, k * 11:(k + 1) * 11],
                        h4[:, 1 + half * TH:1 + (half + 1) * TH, k, :],
                        start=(k == 0), stop=(k == 1),
                    ).then_inc(sem_hd)

        @block.vector
        def _(vector):
            vector.memset(sb_H[:, 0:32], 0.0)
            vector.memset(sb_C[:, 0:32], 0.0).then_inc(sem_h)
            # ---- precompute copies: psum + b_lstm -> XW ----
            vector.wait_ge(dma_bl, 16)
            xw4 = sb_XW[:].rearrange("p (t m b) -> p t m b", m=M_TILES, b=BL)
            idx = 0
            for half in range(n_half):
                for m in range(M_TILES):
                    vector.wait_ge(sem_pre, idx + 1)
                    src = ps_pre[idx % 2][:, 0:HALF].rearrange(
                        "p (t b) -> p t b", b=BL)
                    nc.vector.tensor_scalar_add(
                        xw4[:, half * TH:(half + 1) * TH, m, :], src,
                        sb_blstm[:, m:m + 1],
                    ).then_inc(sem_precp)
                    idx += 1
            # same-engine RAW fence: scan reads XW / C(memset) written above
            vector.drain()
            # ---- scan pointwise ----
            for t in range(T):
                s = t % 2
                zs = ps_zs[s][:, 0:128]
                gs = sb_G[:, s * 128:(s + 1) * 128]
                vector.wait_ge(sem_pe, t + 1)
                nc.vector.tensor_add(zs, ps_z[s][:, 0:128],
                                     sb_XW[:, t * 128:(t + 1) * 128]
                                     ).then_inc(sem_zadd)
                # order c'(t-1) write -> v(t) read of C (cheap same-engine
                # wait instead of a pipeline drain; DVE completes in order)
                if t >= 1:
                    vector.wait_ge(sem_cp, t)
                vector.wait_ge(sem_act, 3 * t + 1)
                nc.vector.tensor_mul(sb_Vt[:], gs[:, 32:64],
                                     sb_C[:, t * 32:(t + 1) * 32])
                vector.wait_ge(sem_act, 3 * t + 2)
                nc.vector.tensor_mul(sb_U[:], gs[:, 0:32], gs[:, 96:128]
                                     ).then_inc(sem_uv)
                # order u/v writes -> c' read (in-order pipe: u done => v done)
                vector.wait_ge(sem_uv, t + 1)
                nc.vector.tensor_add(sb_C[:, (t + 1) * 32:(t + 2) * 32],
                                     sb_U[:], sb_Vt[:]).then_inc(sem_cp)
                vector.wait_ge(sem_act, 3 * t + 3)
                nc.vector.tensor_mul(sb_H[:, (t + 1) * 32:(t + 2) * 32],
                                     gs[:, 64:96],
                                     sb_TC[:, s * 32:(s + 1) * 32]
                                     ).then_inc(sem_h)
            # ---- head copies ----
            vector.wait_ge(dma_bo, 16)
            for half in range(n_half):
                vector.wait_ge(sem_hd, K2 * (half + 1))
                nc.vector.tensor_scalar_add(
                    sb_head[:, half * HALF:(half + 1) * HALF],
                    ps_hd[half][0:11, 0:HALF], sb_bob[0:11, 0:1],
                ).then_inc(sem_hdcp)

        @block.scalar
        def _(scalar):
            Sig = mybir.ActivationFunctionType.Sigmoid
            Tanh = mybir.ActivationFunctionType.Tanh
            for t in range(T):
                s = t % 2
                zs = ps_zs[s][:, 0:128]
                gs = sb_G[:, s * 128:(s + 1) * 128]
                scalar.wait_ge(sem_zadd, t + 1)
                scalar.activation(gs[:, 0:96], zs[:, 0:96], Sig).then_inc(sem_act)
                scalar.activation(gs[:, 96:128], zs[:, 96:128], Tanh
                                  ).then_inc(sem_act)
                scalar.wait_ge(sem_cp, t + 1)
                scalar.activation(sb_TC[:, s * 32:(s + 1) * 32],
                                  sb_C[:, (t + 1) * 32:(t + 2) * 32], Tanh
                                  ).then_inc(sem_act)

    return nc, ctx


_BUILD_CACHE = {}


def _get_nc(T):
    if T not in _BUILD_CACHE:
        _BUILD_CACHE[T] = _build(T)
    return _BUILD_CACHE[T][0]


def _prep_inputs(X, u, Wk, Wr, b_lstm, Wo, bo, Wc, bc, T):
    """Build the 8 per-core input maps (numpy, host-side sharding)."""
    Wk_p = np.ascontiguousarray(Wk[:, GATE_PERM], np.float32)
    Wr_p = Wr[:, GATE_PERM].astype(np.float32)
    WrT = np.ascontiguousarray(
        Wr_p.reshape(2, 128, 1024).transpose(1, 0, 2).reshape(128, 2048)
    ).astype(ml_dtypes.bfloat16)
    blstm = np.ascontiguousarray(
        b_lstm[GATE_PERM].astype(np.float32).reshape(8, 128).T)
    WoC = np.concatenate([Wo.astype(np.float32),
                          Wc[:256].astype(np.float32)], axis=1)  # [256, 11]
    WoC = np.ascontiguousarray(
        WoC.reshape(2, 128, 11).transpose(1, 0, 2).reshape(128, 22)
    ).astype(ml_dtypes.bfloat16)
    bob = np.concatenate([bo.astype(np.float32), [0.0]]).reshape(11, 1)
    bob = np.ascontiguousarray(bob, np.float32)

    in_maps = []
    for i in range(NCORES):
        bsl = slice(i * BL, (i + 1) * BL)
        Xt = np.ascontiguousarray(
            X[bsl, :T, :].astype(np.float32).transpose(2, 1, 0).reshape(128, T * BL))
        in_maps.append({
            "Xt": Xt, "WkT": Wk_p, "WrT": WrT, "blstm": blstm,
            "WoC": WoC, "bob": bob,
        })
    return in_maps


def _sigmoid64(x):
    return 1.0 / (1.0 + np.exp(-x.astype(np.float64)))


def _softmax32(x):
    x = x.astype(np.float32)
    e = np.exp(x - x.max(axis=-1, keepdims=True))
    return (e / e.sum(axis=-1, keepdims=True)).astype(np.float32)


def _fallback_scan(x_seq, u_seq, h0, c0, t0, Wk, Wr, b_lstm, Wo, bo, Wc, bc):
    """Continue the reference recurrence on host for one sample that did not
    halt by t0.  Returns the sample's output row (float32)."""
    h = h0.astype(np.float32).copy()
    c = c0.astype(np.float32).copy()
    Wk = Wk.astype(np.float32); Wr = Wr.astype(np.float32)
    b_lstm = b_lstm.astype(np.float32)
    sig = lambda v: 1.0 / (1.0 + np.exp(-v))
    Tt = x_seq.shape[0]
    logits_last = None
    for t in range(t0, Tt):
        z = x_seq[t] @ Wk + h @ Wr + b_lstm
        i, f, g, o = np.split(z, 4)
        i = sig(i); f = sig(f); g = np.tanh(g); o = sig(o)
        c = f * c + i * g
        h = o * np.tanh(c)
        y = h @ Wo.astype(np.float32) + bo.astype(np.float32)
        logits = _softmax32(y)
        pre = float(h @ Wc[:256, 0].astype(np.float32)) \
            + t * float(Wc[256, 0]) + float(bc[0])
        probs = (1.0 - EPS) * sig(np.float32(pre)) + EPS * 0.05
        if u_seq[t] < probs:
            return logits
        logits_last = logits
    return logits_last


def kernel(**inputs):
    X = np.asarray(inputs["X"], np.float32)
    u = np.asarray(inputs["u"], np.float32)
    Wk = np.asarray(inputs["Wk"], np.float32)
    Wr = np.asarray(inputs["Wr"], np.float32)
    b_lstm = np.asarray(inputs["b_lstm"], np.float32)
    Wo = np.asarray(inputs["Wo"], np.float32)
    bo = np.asarray(inputs["bo"], np.float32)
    Wc = np.asarray(inputs["Wc"], np.float32)
    bc = np.asarray(inputs["bc"], np.float32)
    T = T_EFF

    nc = _get_nc(T)
    in_maps = _prep_inputs(X, u, Wk, Wr, b_lstm, Wo, bo, Wc, bc, T)
    res = run_bass_kernel_spmd(nc, in_maps, list(range(NCORES)))

    wc_t = float(Wc[256, 0])
    bias_c = float(bc[0])
    tvec = np.arange(T, dtype=np.float64)

    out = np.zeros((B, C), np.float32)
    for i in range(NCORES):
        bsl = slice(i * BL, (i + 1) * BL)
        head = res.results[i]["head"]          # [11, T*BL]
        y_pre = head[0:10].reshape(10, T, BL).transpose(1, 2, 0)  # [T, b, 10]
        pre_c = head[10].reshape(T, BL).astype(np.float64)        # [T, b]
        probs = (1.0 - EPS) * _sigmoid64(pre_c + tvec[:, None] * wc_t + bias_c) \
            + EPS * 0.05
        u_core = u[bsl, :T, 0]                 # [b, T]
        a = u_core.T.astype(np.float64) < probs  # [T, b]
        halted = a.any(axis=0)
        tstar = np.argmax(a, axis=0)           # first halt step per sample
        logits = _softmax32(y_pre)             # [T, b, 10]
        for b_ in range(BL):
            if halted[b_]:
                out[i * BL + b_] = logits[tstar[b_], b_]
            else:
                sh = res.results[i]["state_h"].astype(np.float32)
                sc = res.results[i]["state_c"].astype(np.float32)
                h_T = sh.reshape(128, 2, BL).transpose(2, 1, 0) \
                    .reshape(BL, 256)[b_]
                c_T = sc.reshape(128, 2, BL).transpose(2, 1, 0) \
                    .reshape(BL, 256)[b_]
                out[i * BL + b_] = _fallback_scan(
                    X[i * BL + b_], u[i * BL + b_, :, 0], h_T, c_T, T,
                    Wk, Wr, b_lstm, Wo, bo, Wc, bc)
    return out


# revision 10
# speedup vs baseline: 2.7007x; 1.0197x over previous
# Trainium2 Bass kernel for nn_EARLIEST (adaptive-halting LSTM, B=128 T=4096
# V=128 H=256 C=10).
#
# Key observation: the model halts each batch sample at the first step t where
# u[b,t] < probs[b,t], with probs ~= 0.45 early on, so every sample halts
# within a few dozen steps (max 36 for the seed-0 inputs).  The returned
# output only needs logits at each sample's first halt step (or step T-1 for
# never-halted samples).  So the device kernel runs the LSTM scan for only
# T_EFF timesteps, emits pre-softmax logits and the halting dot-product for
# every (t, b), and the host applies the (exact) halting latch.  A numpy
# fallback continues the recurrence from the device's (h, c) state for any
# sample that has not halted by T_EFF (statistically never happens; the
# fallback keeps the kernel correct for arbitrary inputs).
#
# Sharding: data-parallel over batch, 16 samples per core, weights replicated.
# Layout on device is feature-major: h^T is [H=256, b=16] stored as two
# 128-partition k-tiles side by side, so LSTM gate math runs on full
# 128-partition tiles and the recurrent matmuls need no transposes.

import numpy as np
import ml_dtypes

import concourse.bass as bass
import concourse.mybir as mybir
from concourse.bass_utils import run_bass_kernel_spmd

B, T_FULL, V, H, C = 128, 4096, 128, 256, 10
EPS = 0.1
NCORES = 8
BL = B // NCORES  # 16 samples per core
T_EFF = 64
M_TILES = 8   # 4H/128
K2 = 2        # H/128
F32 = mybir.dt.float32
BF16 = mybir.dt.bfloat16

# gate order in z is (i, f, g, o); we permute weight columns to (i, f, o, g)
# so one sigmoid covers a contiguous [i|f|o] block and one tanh covers g.
GATE_PERM = np.concatenate([np.arange(0, 512), np.arange(768, 1024),
                            np.arange(512, 768)])


def _build(T):
    """Build the raw-bass single-core program (SPMD across 8 cores)."""
    nc = bass.Bass()

    d_Xt = nc.dram_tensor("Xt", [128, T * BL], F32, kind="ExternalInput")
    d_WkT = nc.dram_tensor("WkT", [128, 1024], F32, kind="ExternalInput")
    d_WrT = nc.dram_tensor("WrT", [128, 2048], BF16, kind="ExternalInput")
    d_blstm = nc.dram_tensor("blstm", [128, 8], F32, kind="ExternalInput")
    d_WoC = nc.dram_tensor("WoC", [128, 22], BF16, kind="ExternalInput")
    d_bob = nc.dram_tensor("bob", [11, 1], F32, kind="ExternalInput")
    d_head = nc.dram_tensor("head", [11, T * BL], F32, kind="ExternalOutput")
    d_state_h = nc.dram_tensor("state_h", [128, 32], BF16, kind="ExternalOutput")
    d_state_c = nc.dram_tensor("state_c", [128, 32], F32, kind="ExternalOutput")

    NH = T * BL  # head columns
    HALF = NH // 2  # 512 per fp32-matmul moving limit

    from contextlib import ExitStack
    ctx = ExitStack()
    sb_Xt = ctx.enter_context(nc.sbuf_tensor([128, T * BL], F32))
    sb_WkT = ctx.enter_context(nc.sbuf_tensor([128, 1024], F32))
    sb_WrT = ctx.enter_context(nc.sbuf_tensor([128, 2048], BF16))
    sb_blstm = ctx.enter_context(nc.sbuf_tensor([128, 8], F32))
    sb_WoC = ctx.enter_context(nc.sbuf_tensor([128, 22], BF16))
    sb_bob = ctx.enter_context(nc.sbuf_tensor([11, 1], F32))
    sb_XW = ctx.enter_context(nc.sbuf_tensor([128, T * 128], F32))
    sb_H = ctx.enter_context(nc.sbuf_tensor([128, (T + 1) * 32], BF16))
    sb_C = ctx.enter_context(nc.sbuf_tensor([128, (T + 1) * 32], F32))
    sb_Z = ctx.enter_context(nc.sbuf_tensor([128, 2 * 128], F32))
    sb_G = ctx.enter_context(nc.sbuf_tensor([128, 2 * 128], F32))
    sb_TC = ctx.enter_context(nc.sbuf_tensor([128, 2 * 32], F32))
    sb_S = ctx.enter_context(nc.sbuf_tensor([128, 2 * 32], F32))
    sb_U = ctx.enter_context(nc.sbuf_tensor([128, 32], F32))
    sb_Vt = ctx.enter_context(nc.sbuf_tensor([128, 32], F32))
    sb_head = ctx.enter_context(nc.sbuf_tensor([11, T * BL], F32))

    ps_pre = [ctx.enter_context(nc.psum_tensor(f"ps_pre{j}", [128, 512], F32))
              for j in range(2)]
    ps_z = [ctx.enter_context(nc.psum_tensor(f"ps_z{j}", [128, 512], F32))
            for j in range(2)]
    ps_hd = [ctx.enter_context(nc.psum_tensor(f"ps_hd{j}", [128, 512], F32))
             for j in range(2)]
    ps_zs = [ctx.enter_context(nc.psum_tensor(f"ps_zs{j}", [128, 512], F32))
             for j in range(2)]

    # one semaphore per input load: DMA completion order is not program
    # order, so a shared counter would not identify which load finished.
    dma_xt = ctx.enter_context(nc.semaphore("dma_xt"))
    dma_wk = ctx.enter_context(nc.semaphore("dma_wk"))
    dma_wr = ctx.enter_context(nc.semaphore("dma_wr"))
    dma_bl = ctx.enter_context(nc.semaphore("dma_bl"))
    dma_wo = ctx.enter_context(nc.semaphore("dma_wo"))
    dma_bo = ctx.enter_context(nc.semaphore("dma_bo"))
    dma_out = ctx.enter_context(nc.semaphore("dma_out"))
    sem_pre = ctx.enter_context(nc.semaphore("sem_pre"))
    sem_precp = ctx.enter_context(nc.semaphore("sem_precp"))
    sem_h = ctx.enter_context(nc.semaphore("sem_h"))
    sem_zadd = ctx.enter_context(nc.semaphore("sem_zadd"))
    sem_cp = ctx.enter_context(nc.semaphore("sem_cp"))
    sem_act = ctx.enter_context(nc.semaphore("sem_act"))
    sem_pe = ctx.enter_context(nc.semaphore("sem_pe"))
    sem_hd = ctx.enter_context(nc.semaphore("sem_hd"))
    sem_hdcp = ctx.enter_context(nc.semaphore("sem_hdcp"))
    sem_uv = ctx.enter_context(nc.semaphore("sem_uv"))
    sem_cv = ctx.enter_context(nc.semaphore("sem_cv"))

    # number of 32-t halves in the head / precompute split
    n_half = 2
    assert T % n_half == 0
    TH = T // n_half  # t-steps per half (32 for T=64)
    assert TH * BL == HALF

    with nc.Block() as block:

        @block.sync
        def _(sync):
            sync.dma_start(out=sb_Xt[:], in_=d_Xt[:]).then_inc(dma_xt, 16)
            sync.dma_start(out=sb_WkT[:], in_=d_WkT[:]).then_inc(dma_wk, 16)
            sync.dma_start(out=sb_WrT[:], in_=d_WrT[:]).then_inc(dma_wr, 16)
            sync.dma_start(out=sb_blstm[:], in_=d_blstm[:]).then_inc(dma_bl, 16)
            sync.dma_start(out=sb_WoC[:], in_=d_WoC[:]).then_inc(dma_wo, 16)
            sync.dma_start(out=sb_bob[:], in_=d_bob[:]).then_inc(dma_bo, 16)
            sync.wait_ge(sem_hdcp, n_half)
            sync.dma_start(out=d_head[:], in_=sb_head[:]).then_inc(dma_out, 16)
            sync.wait_ge(sem_h, T + 1)
            sync.dma_start(out=d_state_h[:],
                           in_=sb_H[:, T * 32:(T + 1) * 32]).then_inc(dma_out, 16)
            sync.wait_ge(sem_cp, T)
            sync.dma_start(out=d_state_c[:],
                           in_=sb_C[:, T * 32:(T + 1) * 32]).then_inc(dma_out, 16)
            sync.wait_ge(dma_out, 48)

        @block.tensor
        def _(tensor):
            # ---- precompute XW = Wk^T X^T (feature-major) ----
            tensor.wait_ge(dma_xt, 16)
            tensor.wait_ge(dma_wk, 16)
            idx = 0
            for half in range(n_half):
                for m in range(M_TILES):
                    if idx >= 2:
                        tensor.wait_ge(sem_precp, idx - 1)
                    tensor.matmul(
                        ps_pre[idx % 2][:, 0:HALF],
                        sb_WkT[:, m * 128:(m + 1) * 128],
                        sb_Xt[:, half * HALF:(half + 1) * HALF],
                        start=True, stop=True,
                    ).then_inc(sem_pre)
                    idx += 1
            # ---- recurrent scan ----
            tensor.wait_ge(dma_wr, 16)
            for t in range(T):
                tensor.wait_ge(sem_h, t + 1)
                if t >= 2:
                    tensor.wait_ge(sem_zadd, t - 1)
                for m in range(M_TILES):
                    for k in range(K2):
                        mm = tensor.matmul(
                            ps_z[t % 2][:, m * BL:(m + 1) * BL],
                            sb_WrT[:, k * 1024 + m * 128:k * 1024 + (m + 1) * 128],
                            sb_H[:, t * 32 + k * BL:t * 32 + (k + 1) * BL],
                            start=(k == 0), stop=(k == 1),
                        )
                mm.then_inc(sem_pe)
            # ---- head: [Wo | Wc]^T applied to all h_t ----
            tensor.wait_ge(dma_wo, 16)
            tensor.wait_ge(sem_h, T + 1)
            h4 = sb_H[:].rearrange("p (t k b) -> p t k b", k=K2, b=BL)
            for half in range(n_half):
                for k in range(K2):
                    tensor.matmul(
                        ps_hd[half][0:11, 0:HALF],
                        sb_WoC[:, k * 11:(k + 1) * 11],
                        h4[:, 1 + half * TH:1 + (half + 1) * TH, k, :],
                        start=(k == 0), stop=(k == 1),
                    ).then_inc(sem_hd)

        @block.vector
        def _(vector):
            vector.memset(sb_H[:, 0:32], 0.0)
            vector.memset(sb_C[:, 0:32], 0.0).then_inc(sem_h)
            # ---- precompute copies: psum + b_lstm -> XW ----
            vector.wait_ge(dma_bl, 16)
            xw4 = sb_XW[:].rearrange("p (t m b) -> p t m b", m=M_TILES, b=BL)
            idx = 0
            for half in range(n_half):
                for m in range(M_TILES):
                    vector.wait_ge(sem_pre, idx + 1)
                    src = ps_pre[idx % 2][:, 0:HALF].rearrange(
                        "p (t b) -> p t b", b=BL)
                    nc.vector.tensor_scalar_add(
                        xw4[:, half * TH:(half + 1) * TH, m, :], src,
                        sb_blstm[:, m:m + 1],
                    ).then_inc(sem_precp)
                    idx += 1
            # same-engine RAW fence: scan reads XW / C(memset) written above
            vector.drain()
            # ---- scan pointwise ----
            Alu = mybir.AluOpType
            for t in range(T):
                s = t % 2
                zs = ps_zs[s][:, 0:128]
                gs = sb_G[:, s * 128:(s + 1) * 128]
                ss = sb_S[:, s * 32:(s + 1) * 32]
                vector.wait_ge(sem_pe, t + 1)
                nc.vector.tensor_add(zs, ps_z[s][:, 0:128],
                                     sb_XW[:, t * 128:(t + 1) * 128]
                                     ).then_inc(sem_zadd)
                # all gates arrive as tanh(x) (i,f,o weight cols pre-halved on
                # host, so tanh here == 2*sigmoid(orig) - 1).  v2 = (tf+1) (.) c
                # = 2 f (.) c ; u2 = (ti+1) (.) tg = 2 i (.) g ; S = 2 c'.
                # order c(t) write (halve of t-1) -> v2 read of C
                if t >= 1:
                    vector.wait_ge(sem_cv, t)
                vector.wait_ge(sem_act, 2 * t + 1)
                nc.vector.scalar_tensor_tensor(
                    sb_Vt[:], gs[:, 32:64], 1.0, sb_C[:, t * 32:(t + 1) * 32],
                    Alu.add, Alu.mult)
                nc.vector.scalar_tensor_tensor(
                    sb_U[:], gs[:, 0:32], 1.0, gs[:, 96:128],
                    Alu.add, Alu.mult).then_inc(sem_uv)
                # order u/v writes -> S read (in-order pipe: u done => v done)
                vector.wait_ge(sem_uv, t + 1)
                nc.vector.tensor_add(ss, sb_U[:], sb_Vt[:]).then_inc(sem_cp)
                # true cell state for the next step (off critical path; hides
                # under the ACT tanh_c)
                vector.wait_ge(sem_cp, t + 1)
                nc.vector.tensor_scalar_mul(
                    sb_C[:, (t + 1) * 32:(t + 2) * 32], ss, 0.5
                ).then_inc(sem_cv)
                # h2 = (to+1) (.) tanh(c') = 2h; all h-consumers use halved
                # weights on the host side.
                vector.wait_ge(sem_act, 2 * t + 2)
                nc.vector.scalar_tensor_tensor(
                    sb_H[:, (t + 1) * 32:(t + 2) * 32], gs[:, 64:96], 1.0,
                    sb_TC[:, s * 32:(s + 1) * 32], Alu.add, Alu.mult
                ).then_inc(sem_h)
            # ---- head copies ----
            vector.wait_ge(dma_bo, 16)
            for half in range(n_half):
                vector.wait_ge(sem_hd, K2 * (half + 1))
                nc.vector.tensor_scalar_add(
                    sb_head[:, half * HALF:(half + 1) * HALF],
                    ps_hd[half][0:11, 0:HALF], sb_bob[0:11, 0:1],
                ).then_inc(sem_hdcp)

        @block.scalar
        def _(scalar):
            Tanh = mybir.ActivationFunctionType.Tanh
            for t in range(T):
                s = t % 2
                zs = ps_zs[s][:, 0:128]
                gs = sb_G[:, s * 128:(s + 1) * 128]
                scalar.wait_ge(sem_zadd, t + 1)
                scalar.activation(gs[:, 0:128], zs[:, 0:128], Tanh
                                  ).then_inc(sem_act)
                scalar.wait_ge(sem_cp, t + 1)
                scalar.activation(sb_TC[:, s * 32:(s + 1) * 32],
                                  sb_S[:, s * 32:(s + 1) * 32], Tanh,
                                  scale=0.5).then_inc(sem_act)

    return nc, ctx


_BUILD_CACHE = {}


def _get_nc(T):
    if T not in _BUILD_CACHE:
        _BUILD_CACHE[T] = _build(T)
    return _BUILD_CACHE[T][0]


def _prep_inputs(X, u, Wk, Wr, b_lstm, Wo, bo, Wc, bc, T):
    """Build the 8 per-core input maps (numpy, host-side sharding)."""
    # column scaling: i,f,o gates get 0.5 (sigma(x) = (tanh(x/2)+1)/2);
    # row scaling: recurrent/head weights get 0.5 because h is stored as 2h.
    col_scale = np.ones((1, 1024), np.float32)
    col_scale[:, :768] = 0.5          # i, f, o blocks after GATE_PERM
    Wk_p = np.ascontiguousarray(Wk[:, GATE_PERM] * col_scale, np.float32)
    Wr_p = (Wr[:, GATE_PERM].astype(np.float32) * col_scale) * 0.5
    WrT = np.ascontiguousarray(
        Wr_p.reshape(2, 128, 1024).transpose(1, 0, 2).reshape(128, 2048)
    ).astype(ml_dtypes.bfloat16)
    blstm = np.ascontiguousarray(
        (b_lstm[GATE_PERM].astype(np.float32) * col_scale[0]
         ).reshape(8, 128).T)
    WoC = np.concatenate([Wo.astype(np.float32),
                          Wc[:256].astype(np.float32)], axis=1) * 0.5
    WoC = np.ascontiguousarray(
        WoC.reshape(2, 128, 11).transpose(1, 0, 2).reshape(128, 22)
    ).astype(ml_dtypes.bfloat16)
    bob = np.concatenate([bo.astype(np.float32), [0.0]]).reshape(11, 1)
    bob = np.ascontiguousarray(bob, np.float32)

    in_maps = []
    for i in range(NCORES):
        bsl = slice(i * BL, (i + 1) * BL)
        Xt = np.ascontiguousarray(
            X[bsl, :T, :].astype(np.float32).transpose(2, 1, 0).reshape(128, T * BL))
        in_maps.append({
            "Xt": Xt, "WkT": Wk_p, "WrT": WrT, "blstm": blstm,
            "WoC": WoC, "bob": bob,
        })
    return in_maps


def _sigmoid64(x):
    return 1.0 / (1.0 + np.exp(-x.astype(np.float64)))


def _softmax32(x):
    x = x.astype(np.float32)
    e = np.exp(x - x.max(axis=-1, keepdims=True))
    return (e / e.sum(axis=-1, keepdims=True)).astype(np.float32)


def _fallback_scan(x_seq, u_seq, h0, c0, t0, Wk, Wr, b_lstm, Wo, bo, Wc, bc):
    """Continue the reference recurrence on host for one sample that did not
    halt by t0.  Returns the sample's output row (float32)."""
    h = h0.astype(np.float32).copy()
    c = c0.astype(np.float32).copy()
    Wk = Wk.astype(np.float32); Wr = Wr.astype(np.float32)
    b_lstm = b_lstm.astype(np.float32)
    sig = lambda v: 1.0 / (1.0 + np.exp(-v))
    Tt = x_seq.shape[0]
    logits_last = None
    for t in range(t0, Tt):
        z = x_seq[t] @ Wk + h @ Wr + b_lstm
        i, f, g, o = np.split(z, 4)
        i = sig(i); f = sig(f); g = np.tanh(g); o = sig(o)
        c = f * c + i * g
        h = o * np.tanh(c)
        y = h @ Wo.astype(np.float32) + bo.astype(np.float32)
        logits = _softmax32(y)
        pre = float(h @ Wc[:256, 0].astype(np.float32)) \
            + t * float(Wc[256, 0]) + float(bc[0])
        probs = (1.0 - EPS) * sig(np.float32(pre)) + EPS * 0.05
        if u_seq[t] < probs:
            return logits
        logits_last = logits
    return logits_last


def kernel(**inputs):
    X = np.asarray(inputs["X"], np.float32)
    u = np.asarray(inputs["u"], np.float32)
    Wk = np.asarray(inputs["Wk"], np.float32)
    Wr = np.asarray(inputs["Wr"], np.float32)
    b_lstm = np.asarray(inputs["b_lstm"], np.float32)
    Wo = np.asarray(inputs["Wo"], np.float32)
    bo = np.asarray(inputs["bo"], np.float32)
    Wc = np.asarray(inputs["Wc"], np.float32)
    bc = np.asarray(inputs["bc"], np.float32)
    T = T_EFF

    nc = _get_nc(T)
    in_maps = _prep_inputs(X, u, Wk, Wr, b_lstm, Wo, bo, Wc, bc, T)
    res = run_bass_kernel_spmd(nc, in_maps, list(range(NCORES)))

    wc_t = float(Wc[256, 0])
    bias_c = float(bc[0])
    tvec = np.arange(T, dtype=np.float64)

    out = np.zeros((B, C), np.float32)
    for i in range(NCORES):
        bsl = slice(i * BL, (i + 1) * BL)
        head = res.results[i]["head"]          # [11, T*BL]
        y_pre = head[0:10].reshape(10, T, BL).transpose(1, 2, 0)  # [T, b, 10]
        pre_c = head[10].reshape(T, BL).astype(np.float64)        # [T, b]
        probs = (1.0 - EPS) * _sigmoid64(pre_c + tvec[:, None] * wc_t + bias_c) \
            + EPS * 0.05
        u_core = u[bsl, :T, 0]                 # [b, T]
        a = u_core.T.astype(np.float64) < probs  # [T, b]
        halted = a.any(axis=0)
        tstar = np.argmax(a, axis=0)           # first halt step per sample
        logits = _softmax32(y_pre)             # [T, b, 10]
        for b_ in range(BL):
            if halted[b_]:
                out[i * BL + b_] = logits[tstar[b_], b_]
            else:
                sh = res.results[i]["state_h"].astype(np.float32) * 0.5
                sc = res.results[i]["state_c"].astype(np.float32)
                h_T = sh.reshape(128, 2, BL).transpose(2, 1, 0) \
                    .reshape(BL, 256)[b_]
                c_T = sc.reshape(128, 2, BL).transpose(2, 1, 0) \
                    .reshape(BL, 256)[b_]
                out[i * BL + b_] = _fallback_scan(
                    X[i * BL + b_], u[i * BL + b_, :, 0], h_T, c_T, T,
                    Wk, Wr, b_lstm, Wo, bo, Wc, bc)
    return out


# revision 11
# speedup vs baseline: 3.6036x; 1.3344x over previous
# Trainium2 Bass kernel for nn_EARLIEST (adaptive-halting LSTM, B=128 T=4096
# V=128 H=256 C=10).
#
# Key observation: the model halts each batch sample at the first step t where
# u[b,t] < probs[b,t], with probs ~= 0.45 early on, so every sample halts
# within a few dozen steps (max 36 for the seed-0 inputs).  The returned
# output only needs logits at each sample's first halt step (or step T-1 for
# never-halted samples).  So the device kernel runs the LSTM scan for only
# T_EFF timesteps, emits pre-softmax logits and the halting dot-product for
# every (t, b), and the host applies the (exact) halting latch.  A numpy
# fallback continues the recurrence from the device's (h, c) state for any
# sample that has not halted by T_EFF (statistically never happens; the
# fallback keeps the kernel correct for arbitrary inputs).
#
# Sharding: data-parallel over batch, 16 samples per core, weights replicated.
# Layout on device is feature-major: h^T is [H=256, b=16] stored as two
# 128-partition k-tiles side by side, so LSTM gate math runs on full
# 128-partition tiles and the recurrent matmuls need no transposes.

import numpy as np
import ml_dtypes

import concourse.bass as bass
import concourse.mybir as mybir
from concourse.bass_utils import run_bass_kernel_spmd

B, T_FULL, V, H, C = 128, 4096, 128, 256, 10
EPS = 0.1
NCORES = 8
BL = B // NCORES  # 16 samples per core
T_EFF = 48
M_TILES = 8   # 4H/128
K2 = 2        # H/128
F32 = mybir.dt.float32
BF16 = mybir.dt.bfloat16

# gate order in z is (i, f, g, o); we permute weight columns to (i, f, o, g)
# so one sigmoid covers a contiguous [i|f|o] block and one tanh covers g.
GATE_PERM = np.concatenate([np.arange(0, 512), np.arange(768, 1024),
                            np.arange(512, 768)])


def _build(T):
    """Build the raw-bass single-core program (SPMD across 8 cores)."""
    nc = bass.Bass()

    d_Xt = nc.dram_tensor("Xt", [128, T * BL], F32, kind="ExternalInput")
    d_WkT = nc.dram_tensor("WkT", [128, 1024], F32, kind="ExternalInput")
    d_WrT = nc.dram_tensor("WrT", [128, 2048], BF16, kind="ExternalInput")
    d_blstm = nc.dram_tensor("blstm", [128, 8], F32, kind="ExternalInput")
    d_WoC = nc.dram_tensor("WoC", [128, 22], BF16, kind="ExternalInput")
    d_bob = nc.dram_tensor("bob", [11, 1], F32, kind="ExternalInput")
    d_head = nc.dram_tensor("head", [11, T * BL], F32, kind="ExternalOutput")
    d_state_h = nc.dram_tensor("state_h", [128, 32], BF16, kind="ExternalOutput")
    d_state_c = nc.dram_tensor("state_c", [128, 32], F32, kind="ExternalOutput")

    NH = T * BL  # head columns
    HALF = NH // 2  # 512 per fp32-matmul moving limit

    from contextlib import ExitStack
    ctx = ExitStack()
    sb_Xt = ctx.enter_context(nc.sbuf_tensor([128, T * BL], F32))
    sb_WkT = ctx.enter_context(nc.sbuf_tensor([128, 1024], F32))
    sb_WrT = ctx.enter_context(nc.sbuf_tensor([128, 2048], BF16))
    sb_blstm = ctx.enter_context(nc.sbuf_tensor([128, 8], F32))
    sb_WoC = ctx.enter_context(nc.sbuf_tensor([128, 22], BF16))
    sb_bob = ctx.enter_context(nc.sbuf_tensor([11, 1], F32))
    sb_XW = ctx.enter_context(nc.sbuf_tensor([128, T * 128], F32))
    sb_H = ctx.enter_context(nc.sbuf_tensor([128, (T + 1) * 32], BF16))
    sb_C = ctx.enter_context(nc.sbuf_tensor([128, (T + 1) * 32], F32))
    sb_Z = ctx.enter_context(nc.sbuf_tensor([128, 2 * 128], F32))
    sb_G = ctx.enter_context(nc.sbuf_tensor([128, 2 * 128], F32))
    sb_TC = ctx.enter_context(nc.sbuf_tensor([128, 2 * 32], F32))
    sb_S = ctx.enter_context(nc.sbuf_tensor([128, 2 * 32], F32))
    sb_U = ctx.enter_context(nc.sbuf_tensor([128, 32], F32))
    sb_Vt = ctx.enter_context(nc.sbuf_tensor([128, 32], F32))
    sb_head = ctx.enter_context(nc.sbuf_tensor([11, T * BL], F32))

    ps_pre = [ctx.enter_context(nc.psum_tensor(f"ps_pre{j}", [128, 512], F32))
              for j in range(2)]
    ps_z = [ctx.enter_context(nc.psum_tensor(f"ps_z{j}", [128, 512], F32))
            for j in range(2)]
    ps_hd = [ctx.enter_context(nc.psum_tensor(f"ps_hd{j}", [128, 512], F32))
             for j in range(2)]
    ps_zs = [ctx.enter_context(nc.psum_tensor(f"ps_zs{j}", [128, 512], F32))
             for j in range(2)]

    # one semaphore per input load: DMA completion order is not program
    # order, so a shared counter would not identify which load finished.
    dma_xt = ctx.enter_context(nc.semaphore("dma_xt"))
    dma_wk = ctx.enter_context(nc.semaphore("dma_wk"))
    dma_wr = ctx.enter_context(nc.semaphore("dma_wr"))
    dma_bl = ctx.enter_context(nc.semaphore("dma_bl"))
    dma_wo = ctx.enter_context(nc.semaphore("dma_wo"))
    dma_bo = ctx.enter_context(nc.semaphore("dma_bo"))
    dma_out = ctx.enter_context(nc.semaphore("dma_out"))
    sem_pre = ctx.enter_context(nc.semaphore("sem_pre"))
    sem_precp = ctx.enter_context(nc.semaphore("sem_precp"))
    sem_h = ctx.enter_context(nc.semaphore("sem_h"))
    sem_zadd = ctx.enter_context(nc.semaphore("sem_zadd"))
    sem_cp = ctx.enter_context(nc.semaphore("sem_cp"))
    sem_act = ctx.enter_context(nc.semaphore("sem_act"))
    sem_pe = ctx.enter_context(nc.semaphore("sem_pe"))
    sem_hd = ctx.enter_context(nc.semaphore("sem_hd"))
    sem_hdcp = ctx.enter_context(nc.semaphore("sem_hdcp"))
    sem_uv = ctx.enter_context(nc.semaphore("sem_uv"))
    sem_cv = ctx.enter_context(nc.semaphore("sem_cv"))

    # number of 32-t halves in the head / precompute split
    n_half = 2
    assert T % n_half == 0
    TH = T // n_half  # t-steps per half (32 for T=64)
    assert TH * BL == HALF

    with nc.Block() as block:

        @block.sync
        def _(sync):
            sync.dma_start(out=sb_Xt[:], in_=d_Xt[:]).then_inc(dma_xt, 16)
            sync.dma_start(out=sb_WkT[:], in_=d_WkT[:]).then_inc(dma_wk, 16)
            sync.dma_start(out=sb_WrT[:], in_=d_WrT[:]).then_inc(dma_wr, 16)
            sync.dma_start(out=sb_blstm[:], in_=d_blstm[:]).then_inc(dma_bl, 16)
            sync.dma_start(out=sb_WoC[:], in_=d_WoC[:]).then_inc(dma_wo, 16)
            sync.dma_start(out=sb_bob[:], in_=d_bob[:]).then_inc(dma_bo, 16)
            sync.wait_ge(sem_hdcp, n_half)
            sync.dma_start(out=d_head[:], in_=sb_head[:]).then_inc(dma_out, 16)
            sync.wait_ge(sem_h, T + 1)
            sync.dma_start(out=d_state_h[:],
                           in_=sb_H[:, T * 32:(T + 1) * 32]).then_inc(dma_out, 16)
            sync.wait_ge(sem_cp, T)
            sync.dma_start(out=d_state_c[:],
                           in_=sb_C[:, T * 32:(T + 1) * 32]).then_inc(dma_out, 16)
            sync.wait_ge(dma_out, 48)

        @block.tensor
        def _(tensor):
            # ---- precompute XW = Wk^T X^T (feature-major) ----
            # half 0 (t < T/2) runs up front; half 1 is interleaved one
            # matmul per scan step into PE's idle window during the tail.
            def pre_mm(idx):
                half, m = divmod(idx, M_TILES)
                if idx >= 2:
                    tensor.wait_ge(sem_precp, idx - 1)
                tensor.matmul(
                    ps_pre[idx % 2][:, 0:HALF],
                    sb_WkT[:, m * 128:(m + 1) * 128],
                    sb_Xt[:, half * HALF:(half + 1) * HALF],
                    start=True, stop=True,
                ).then_inc(sem_pre)

            tensor.wait_ge(dma_xt, 16)
            tensor.wait_ge(dma_wk, 16)
            for idx in range(M_TILES):
                pre_mm(idx)
            # ---- recurrent scan ----
            tensor.wait_ge(dma_wr, 16)
            for t in range(T):
                tensor.wait_ge(sem_h, t + 1)
                if t >= 2:
                    tensor.wait_ge(sem_zadd, t - 1)
                for m in range(M_TILES):
                    for k in range(K2):
                        mm = tensor.matmul(
                            ps_z[t % 2][:, m * BL:(m + 1) * BL],
                            sb_WrT[:, k * 1024 + m * 128:k * 1024 + (m + 1) * 128],
                            sb_H[:, t * 32 + k * BL:t * 32 + (k + 1) * BL],
                            start=(k == 0), stop=(k == 1),
                        )
                mm.then_inc(sem_pe)
                if t < M_TILES:
                    pre_mm(M_TILES + t)
            # ---- head: [Wo | Wc]^T applied to all h_t ----
            tensor.wait_ge(dma_wo, 16)
            tensor.wait_ge(sem_h, T + 1)
            h4 = sb_H[:].rearrange("p (t k b) -> p t k b", k=K2, b=BL)
            for half in range(n_half):
                for k in range(K2):
                    tensor.matmul(
                        ps_hd[half][0:11, 0:HALF],
                        sb_WoC[:, k * 11:(k + 1) * 11],
                        h4[:, 1 + half * TH:1 + (half + 1) * TH, k, :],
                        start=(k == 0), stop=(k == 1),
                    ).then_inc(sem_hd)

        @block.vector
        def _(vector):
            vector.memset(sb_H[:, 0:32], 0.0)
            vector.memset(sb_C[:, 0:32], 0.0).then_inc(sem_h)
            # ---- precompute copies: psum + b_lstm -> XW ----
            vector.wait_ge(dma_bl, 16)
            xw4 = sb_XW[:].rearrange("p (t m b) -> p t m b", m=M_TILES, b=BL)

            def pre_copy(idx):
                half, m = divmod(idx, M_TILES)
                vector.wait_ge(sem_pre, idx + 1)
                psrc = ps_pre[idx % 2][:, 0:HALF].rearrange(
                    "p (t b) -> p t b", b=BL)
                nc.vector.tensor_scalar_add(
                    xw4[:, half * TH:(half + 1) * TH, m, :], psrc,
                    sb_blstm[:, m:m + 1],
                ).then_inc(sem_precp)

            for idx in range(M_TILES):
                pre_copy(idx)
            # same-engine RAW fence: scan reads XW / C(memset) written above
            vector.drain()
            # ---- scan pointwise ----
            Alu = mybir.AluOpType
            for t in range(T):
                s = t % 2
                zs = ps_zs[s][:, 0:128]
                gs = sb_G[:, s * 128:(s + 1) * 128]
                ss = sb_S[:, s * 32:(s + 1) * 32]
                vector.wait_ge(sem_pe, t + 1)
                if t == TH:
                    # half-1 XW written by interleaved same-engine copies
                    vector.wait_ge(sem_precp, 2 * M_TILES)
                nc.vector.tensor_add(zs, ps_z[s][:, 0:128],
                                     sb_XW[:, t * 128:(t + 1) * 128]
                                     ).then_inc(sem_zadd)
                # all gates arrive as tanh(x) (i,f,o weight cols pre-halved on
                # host, so tanh here == 2*sigmoid(orig) - 1).  v2 = (tf+1) (.) c
                # = 2 f (.) c ; u2 = (ti+1) (.) tg = 2 i (.) g ; S = 2 c'.
                # order c(t) write (halve of t-1) -> v2 read of C
                if t >= 1:
                    vector.wait_ge(sem_cv, t)
                vector.wait_ge(sem_act, 2 * t + 1)
                nc.vector.scalar_tensor_tensor(
                    sb_Vt[:], gs[:, 32:64], 1.0, sb_C[:, t * 32:(t + 1) * 32],
                    Alu.add, Alu.mult)
                nc.vector.scalar_tensor_tensor(
                    sb_U[:], gs[:, 0:32], 1.0, gs[:, 96:128],
                    Alu.add, Alu.mult).then_inc(sem_uv)
                # order u/v writes -> S read (in-order pipe: u done => v done)
                vector.wait_ge(sem_uv, t + 1)
                nc.vector.tensor_add(ss, sb_U[:], sb_Vt[:]).then_inc(sem_cp)
                # true cell state for the next step (off critical path; hides
                # under the ACT tanh_c)
                vector.wait_ge(sem_cp, t + 1)
                nc.vector.tensor_scalar_mul(
                    sb_C[:, (t + 1) * 32:(t + 2) * 32], ss, 0.5
                ).then_inc(sem_cv)
                # h2 = (to+1) (.) tanh(c') = 2h; all h-consumers use halved
                # weights on the host side.
                vector.wait_ge(sem_act, 2 * t + 2)
                nc.vector.scalar_tensor_tensor(
                    sb_H[:, (t + 1) * 32:(t + 2) * 32], gs[:, 64:96], 1.0,
                    sb_TC[:, s * 32:(s + 1) * 32], Alu.add, Alu.mult
                ).then_inc(sem_h)
                if t < M_TILES:
                    pre_copy(M_TILES + t)
            # ---- head copies ----
            vector.wait_ge(dma_bo, 16)
            for half in range(n_half):
                vector.wait_ge(sem_hd, K2 * (half + 1))
                nc.vector.tensor_scalar_add(
                    sb_head[:, half * HALF:(half + 1) * HALF],
                    ps_hd[half][0:11, 0:HALF], sb_bob[0:11, 0:1],
                ).then_inc(sem_hdcp)

        @block.scalar
        def _(scalar):
            Tanh = mybir.ActivationFunctionType.Tanh
            for t in range(T):
                s = t % 2
                zs = ps_zs[s][:, 0:128]
                gs = sb_G[:, s * 128:(s + 1) * 128]
                scalar.wait_ge(sem_zadd, t + 1)
                scalar.activation(gs[:, 0:128], zs[:, 0:128], Tanh
                                  ).then_inc(sem_act)
                scalar.wait_ge(sem_cp, t + 1)
                scalar.activation(sb_TC[:, s * 32:(s + 1) * 32],
                                  sb_S[:, s * 32:(s + 1) * 32], Tanh,
                                  scale=0.5).then_inc(sem_act)

    return nc, ctx


_BUILD_CACHE = {}


def _get_nc(T):
    if T not in _BUILD_CACHE:
        _BUILD_CACHE[T] = _build(T)
    return _BUILD_CACHE[T][0]


def _prep_inputs(X, u, Wk, Wr, b_lstm, Wo, bo, Wc, bc, T):
    """Build the 8 per-core input maps (numpy, host-side sharding)."""
    # column scaling: i,f,o gates get 0.5 (sigma(x) = (tanh(x/2)+1)/2);
    # row scaling: recurrent/head weights get 0.5 because h is stored as 2h.
    col_scale = np.ones((1, 1024), np.float32)
    col_scale[:, :768] = 0.5          # i, f, o blocks after GATE_PERM
    Wk_p = np.ascontiguousarray(Wk[:, GATE_PERM] * col_scale, np.float32)
    Wr_p = (Wr[:, GATE_PERM].astype(np.float32) * col_scale) * 0.5
    WrT = np.ascontiguousarray(
        Wr_p.reshape(2, 128, 1024).transpose(1, 0, 2).reshape(128, 2048)
    ).astype(ml_dtypes.bfloat16)
    blstm = np.ascontiguousarray(
        (b_lstm[GATE_PERM].astype(np.float32) * col_scale[0]
         ).reshape(8, 128).T)
    WoC = np.concatenate([Wo.astype(np.float32),
                          Wc[:256].astype(np.float32)], axis=1) * 0.5
    WoC = np.ascontiguousarray(
        WoC.reshape(2, 128, 11).transpose(1, 0, 2).reshape(128, 22)
    ).astype(ml_dtypes.bfloat16)
    bob = np.concatenate([bo.astype(np.float32), [0.0]]).reshape(11, 1)
    bob = np.ascontiguousarray(bob, np.float32)

    in_maps = []
    for i in range(NCORES):
        bsl = slice(i * BL, (i + 1) * BL)
        Xt = np.ascontiguousarray(
            X[bsl, :T, :].astype(np.float32).transpose(2, 1, 0).reshape(128, T * BL))
        in_maps.append({
            "Xt": Xt, "WkT": Wk_p, "WrT": WrT, "blstm": blstm,
            "WoC": WoC, "bob": bob,
        })
    return in_maps


def _sigmoid64(x):
    return 1.0 / (1.0 + np.exp(-x.astype(np.float64)))


def _softmax32(x):
    x = x.astype(np.float32)
    e = np.exp(x - x.max(axis=-1, keepdims=True))
    return (e / e.sum(axis=-1, keepdims=True)).astype(np.float32)


def _fallback_scan(x_seq, u_seq, h0, c0, t0, Wk, Wr, b_lstm, Wo, bo, Wc, bc):
    """Continue the reference recurrence on host for one sample that did not
    halt by t0.  Returns the sample's output row (float32)."""
    h = h0.astype(np.float32).copy()
    c = c0.astype(np.float32).copy()
    Wk = Wk.astype(np.float32); Wr = Wr.astype(np.float32)
    b_lstm = b_lstm.astype(np.float32)
    sig = lambda v: 1.0 / (1.0 + np.exp(-v))
    Tt = x_seq.shape[0]
    logits_last = None
    for t in range(t0, Tt):
        z = x_seq[t] @ Wk + h @ Wr + b_lstm
        i, f, g, o = np.split(z, 4)
        i = sig(i); f = sig(f); g = np.tanh(g); o = sig(o)
        c = f * c + i * g
        h = o * np.tanh(c)
        y = h @ Wo.astype(np.float32) + bo.astype(np.float32)
        logits = _softmax32(y)
        pre = float(h @ Wc[:256, 0].astype(np.float32)) \
            + t * float(Wc[256, 0]) + float(bc[0])
        probs = (1.0 - EPS) * sig(np.float32(pre)) + EPS * 0.05
        if u_seq[t] < probs:
            return logits
        logits_last = logits
    return logits_last


def kernel(**inputs):
    X = np.asarray(inputs["X"], np.float32)
    u = np.asarray(inputs["u"], np.float32)
    Wk = np.asarray(inputs["Wk"], np.float32)
    Wr = np.asarray(inputs["Wr"], np.float32)
    b_lstm = np.asarray(inputs["b_lstm"], np.float32)
    Wo = np.asarray(inputs["Wo"], np.float32)
    bo = np.asarray(inputs["bo"], np.float32)
    Wc = np.asarray(inputs["Wc"], np.float32)
    bc = np.asarray(inputs["bc"], np.float32)
    T = T_EFF

    nc = _get_nc(T)
    in_maps = _prep_inputs(X, u, Wk, Wr, b_lstm, Wo, bo, Wc, bc, T)
    res = run_bass_kernel_spmd(nc, in_maps, list(range(NCORES)))

    wc_t = float(Wc[256, 0])
    bias_c = float(bc[0])
    tvec = np.arange(T, dtype=np.float64)

    out = np.zeros((B, C), np.float32)
    for i in range(NCORES):
        bsl = slice(i * BL, (i + 1) * BL)
        head = res.results[i]["head"]          # [11, T*BL]
        y_pre = head[0:10].reshape(10, T, BL).transpose(1, 2, 0)  # [T, b, 10]
        pre_c = head[10].reshape(T, BL).astype(np.float64)        # [T, b]
        probs = (1.0 - EPS) * _sigmoid64(pre_c + tvec[:, None] * wc_t + bias_c) \
            + EPS * 0.05
        u_core = u[bsl, :T, 0]                 # [b, T]
        a = u_core.T.astype(np.float64) < probs  # [T, b]
        halted = a.any(axis=0)
        tstar = np.argmax(a, axis=0)           # first halt step per sample
        logits = _softmax32(y_pre)             # [T, b, 10]
        for b_ in range(BL):
            if halted[b_]:
                out[i * BL + b_] = logits[tstar[b_], b_]
            else:
                sh = res.results[i]["state_h"].astype(np.float32) * 0.5
                sc = res.results[i]["state_c"].astype(np.float32)
                h_T = sh.reshape(128, 2, BL).transpose(2, 1, 0) \
                    .reshape(BL, 256)[b_]
                c_T = sc.reshape(128, 2, BL).transpose(2, 1, 0) \
                    .reshape(BL, 256)[b_]
                out[i * BL + b_] = _fallback_scan(
                    X[i * BL + b_], u[i * BL + b_, :, 0], h_T, c_T, T,
                    Wk, Wr, b_lstm, Wo, bo, Wc, bc)
    return out


# revision 14
# speedup vs baseline: 3.9815x; 1.1049x over previous
# Trainium2 Bass kernel for nn_EARLIEST (adaptive-halting LSTM, B=128 T=4096
# V=128 H=256 C=10).
#
# Key observation: the model halts each batch sample at the first step t where
# u[b,t] < probs[b,t], with probs ~= 0.45 early on, so every sample halts
# within a few dozen steps (max 36 for the seed-0 inputs).  The returned
# output only needs logits at each sample's first halt step (or step T-1 for
# never-halted samples).  So the device kernel runs the LSTM scan for only
# T_EFF timesteps, emits pre-softmax logits and the halting dot-product for
# every (t, b), and the host applies the (exact) halting latch.  A numpy
# fallback continues the recurrence from the device's (h, c) state for any
# sample that has not halted by T_EFF (statistically never happens; the
# fallback keeps the kernel correct for arbitrary inputs).
#
# Sharding: data-parallel over batch, 16 samples per core, weights replicated.
# Layout on device is feature-major: h^T is [H=256, b=16] stored as two
# 128-partition k-tiles side by side, so LSTM gate math runs on full
# 128-partition tiles and the recurrent matmuls need no transposes.

import numpy as np
import ml_dtypes

import concourse.bass as bass
import concourse.mybir as mybir
from concourse.bass_utils import run_bass_kernel_spmd

B, T_FULL, V, H, C = 128, 4096, 128, 256, 10
EPS = 0.1
NCORES = 8
BL = B // NCORES  # 16 samples per core
T_EFF = 48
M_TILES = 8   # 4H/128
K2 = 2        # H/128
F32 = mybir.dt.float32
F16 = mybir.dt.float16

# gate order in z is (i, f, g, o); we permute weight columns to (i, f, o, g)
# so one sigmoid covers a contiguous [i|f|o] block and one tanh covers g.
GATE_PERM = np.concatenate([np.arange(0, 512), np.arange(768, 1024),
                            np.arange(512, 768)])


def _build(T):
    """Build the raw-bass single-core program (SPMD across 8 cores)."""
    nc = bass.Bass()

    d_Xt = nc.dram_tensor("Xt", [128, T * BL], F16, kind="ExternalInput")
    d_WkT = nc.dram_tensor("WkT", [128, 1024], F16, kind="ExternalInput")
    d_WrT = nc.dram_tensor("WrT", [128, 2048], F16, kind="ExternalInput")
    d_ident = nc.dram_tensor("ident", [128, 128], F16, kind="ExternalInput")
    d_blstm = nc.dram_tensor("blstm", [128, 8], F32, kind="ExternalInput")
    d_WoC = nc.dram_tensor("WoC", [128, 22], F16, kind="ExternalInput")
    d_bob = nc.dram_tensor("bob", [11, 1], F32, kind="ExternalInput")
    d_head = nc.dram_tensor("head", [11, T * BL], F32, kind="ExternalOutput")
    d_state_h = nc.dram_tensor("state_h", [128, 32], F16, kind="ExternalOutput")
    d_state_c = nc.dram_tensor("state_c", [128, 32], F32, kind="ExternalOutput")

    NH = T * BL
    HALF = NH // 2  # fp32 head matmul moving-operand limit is 512

    from contextlib import ExitStack
    ctx = ExitStack()
    sb_Xt = ctx.enter_context(nc.sbuf_tensor([128, T * BL], F16))
    sb_WkT = ctx.enter_context(nc.sbuf_tensor([128, 1024], F16))
    sb_WrT = ctx.enter_context(nc.sbuf_tensor([128, 2048], F16))
    sb_I = ctx.enter_context(nc.sbuf_tensor([128, 128], F16))
    sb_blstm = ctx.enter_context(nc.sbuf_tensor([128, 8], F32))
    sb_WoC = ctx.enter_context(nc.sbuf_tensor([128, 22], F16))
    sb_bob = ctx.enter_context(nc.sbuf_tensor([11, 1], F32))
    sb_XW = ctx.enter_context(nc.sbuf_tensor([128, T * 128], F16))
    sb_H = ctx.enter_context(nc.sbuf_tensor([128, (T + 1) * 32], F16))
    sb_C = ctx.enter_context(nc.sbuf_tensor([128, (T + 1) * 32], F32))
    sb_G = ctx.enter_context(nc.sbuf_tensor([128, 2 * 128], F32))
    sb_TC = ctx.enter_context(nc.sbuf_tensor([128, 2 * 32], F32))
    sb_S = ctx.enter_context(nc.sbuf_tensor([128, 2 * 32], F32))
    sb_U = ctx.enter_context(nc.sbuf_tensor([128, 32], F32))
    sb_Vt = ctx.enter_context(nc.sbuf_tensor([128, 32], F32))
    sb_head = ctx.enter_context(nc.sbuf_tensor([11, T * BL], F32))

    ps_pre = [ctx.enter_context(nc.psum_tensor(f"ps_pre{j}", [128, 512], F32))
              for j in range(2)]
    ps_z = [ctx.enter_context(nc.psum_tensor(f"ps_z{j}", [128, 512], F32))
            for j in range(2)]
    ps_hd = [ctx.enter_context(nc.psum_tensor(f"ps_hd{j}", [128, 512], F32))
             for j in range(2)]

    # one semaphore per input load: DMA completion order is not program order
    dma_xt = ctx.enter_context(nc.semaphore("dma_xt"))
    dma_wk = ctx.enter_context(nc.semaphore("dma_wk"))
    dma_wr = ctx.enter_context(nc.semaphore("dma_wr"))
    dma_id = ctx.enter_context(nc.semaphore("dma_id"))
    dma_bl = ctx.enter_context(nc.semaphore("dma_bl"))
    dma_wo = ctx.enter_context(nc.semaphore("dma_wo"))
    dma_bo = ctx.enter_context(nc.semaphore("dma_bo"))
    dma_out = ctx.enter_context(nc.semaphore("dma_out"))
    sem_pre = ctx.enter_context(nc.semaphore("sem_pre"))
    sem_precp = ctx.enter_context(nc.semaphore("sem_precp"))
    sem_h = ctx.enter_context(nc.semaphore("sem_h"))
    sem_cp = ctx.enter_context(nc.semaphore("sem_cp"))
    sem_act = ctx.enter_context(nc.semaphore("sem_act"))
    sem_pe = ctx.enter_context(nc.semaphore("sem_pe"))
    sem_hd = ctx.enter_context(nc.semaphore("sem_hd"))
    sem_hdcp = ctx.enter_context(nc.semaphore("sem_hdcp"))
    sem_uv = ctx.enter_context(nc.semaphore("sem_uv"))
    sem_cv = ctx.enter_context(nc.semaphore("sem_cv"))

    n_half = 2
    assert T % n_half == 0
    TH = T // n_half
    assert TH * BL == HALF

    with nc.Block() as block:

        @block.sync
        def _(sync):
            sync.dma_start(out=sb_Xt[:], in_=d_Xt[:]).then_inc(dma_xt, 16)
            sync.dma_start(out=sb_WkT[:], in_=d_WkT[:]).then_inc(dma_wk, 16)
            sync.dma_start(out=sb_WrT[:], in_=d_WrT[:]).then_inc(dma_wr, 16)
            sync.dma_start(out=sb_I[:], in_=d_ident[:]).then_inc(dma_id, 16)
            sync.dma_start(out=sb_blstm[:], in_=d_blstm[:]).then_inc(dma_bl, 16)
            sync.dma_start(out=sb_WoC[:], in_=d_WoC[:]).then_inc(dma_wo, 16)
            sync.dma_start(out=sb_bob[:], in_=d_bob[:]).then_inc(dma_bo, 16)
            sync.wait_ge(sem_hdcp, n_half)
            sync.dma_start(out=d_head[:], in_=sb_head[:]).then_inc(dma_out, 16)
            sync.wait_ge(sem_h, T + 1)
            sync.dma_start(out=d_state_h[:],
                           in_=sb_H[:, T * 32:(T + 1) * 32]).then_inc(dma_out, 16)
            sync.wait_ge(sem_cv, T)
            sync.dma_start(out=d_state_c[:],
                           in_=sb_C[:, T * 32:(T + 1) * 32]).then_inc(dma_out, 16)
            sync.wait_ge(dma_out, 48)

        @block.tensor
        def _(tensor):
            # ---- precompute XW = Wk^T X^T (feature-major, fp16) ----
            # half 0 runs up front; half 1 is interleaved into the scan.
            def pre_mm(idx):
                half, m = divmod(idx, M_TILES)
                if idx >= 2:
                    tensor.wait_ge(sem_precp, idx - 1)
                tensor.matmul(
                    ps_pre[idx % 2][:, 0:HALF],
                    sb_WkT[:, m * 128:(m + 1) * 128],
                    sb_Xt[:, half * HALF:(half + 1) * HALF],
                    start=True, stop=True,
                ).then_inc(sem_pre)

            tensor.wait_ge(dma_xt, 16)
            tensor.wait_ge(dma_wk, 16)
            for idx in range(M_TILES):
                pre_mm(idx)
            # ---- recurrent scan ----
            tensor.wait_ge(dma_wr, 16)
            tensor.wait_ge(dma_id, 16)
            tensor.wait_ge(sem_precp, M_TILES)   # XW half 0 in SBUF
            for t in range(T):
                if t == TH:
                    tensor.wait_ge(sem_precp, 2 * M_TILES)  # XW half 1
                tensor.wait_ge(sem_h, t + 1)
                if t >= 2:
                    # ps_z bank reuse: ACT consumed step t-2 gates
                    tensor.wait_ge(sem_act, 2 * (t - 2) + 1)
                # prefill z with XW[t] in one identity matmul, then
                # accumulate the recurrent part on top.
                tensor.matmul(ps_z[t % 2][:, 0:128], sb_I[:],
                              sb_XW[:, t * 128:(t + 1) * 128],
                              start=True, stop=True, skip_group_check=True)
                for m in range(M_TILES):
                    for k in range(K2):
                        mm = tensor.matmul(
                            ps_z[t % 2][:, m * BL:(m + 1) * BL],
                            sb_WrT[:, k * 1024 + m * 128:k * 1024 + (m + 1) * 128],
                            sb_H[:, t * 32 + k * BL:t * 32 + (k + 1) * BL],
                            start=False, stop=False, skip_group_check=True,
                        )
                mm.then_inc(sem_pe)
                if t < M_TILES:
                    pre_mm(M_TILES + t)
            # ---- head: [Wo | Wc]^T applied to all h_t ----
            tensor.wait_ge(dma_wo, 16)
            tensor.wait_ge(sem_h, T + 1)
            h4 = sb_H[:].rearrange("p (t k b) -> p t k b", k=K2, b=BL)
            for half in range(n_half):
                for k in range(K2):
                    tensor.matmul(
                        ps_hd[half][0:11, 0:HALF],
                        sb_WoC[:, k * 11:(k + 1) * 11],
                        h4[:, 1 + half * TH:1 + (half + 1) * TH, k, :],
                        start=(k == 0), stop=(k == 1),
                    ).then_inc(sem_hd)

        @block.vector
        def _(vector):
            vector.memset(sb_H[:, 0:32], 0.0)
            vector.memset(sb_C[:, 0:32], 0.0).then_inc(sem_h)
            # ---- precompute copies: psum + b_lstm -> XW (fp16) ----
            vector.wait_ge(dma_bl, 16)
            xw4 = sb_XW[:].rearrange("p (t m b) -> p t m b", m=M_TILES, b=BL)

            def pre_copy(idx):
                half, m = divmod(idx, M_TILES)
                vector.wait_ge(sem_pre, idx + 1)
                psrc = ps_pre[idx % 2][:, 0:HALF].rearrange(
                    "p (t b) -> p t b", b=BL)
                nc.vector.tensor_scalar_add(
                    xw4[:, half * TH:(half + 1) * TH, m, :], psrc,
                    sb_blstm[:, m:m + 1],
                ).then_inc(sem_precp)

            for idx in range(M_TILES):
                pre_copy(idx)
            # same-engine fence: v2(0) reads C written by memset above
            vector.drain()
            # ---- scan pointwise ----
            Alu = mybir.AluOpType
            for t in range(T):
                s = t % 2
                gs = sb_G[:, s * 128:(s + 1) * 128]
                ss = sb_S[:, s * 32:(s + 1) * 32]
                # all gates arrive as tanh (i,f,o weight cols pre-halved on
                # host): v2 = (tf+1)(.)c = 2f(.)c ; u2 = (ti+1)(.)tg ; S = 2c'
                if t >= 1:
                    vector.wait_ge(sem_cv, t)  # c(t) committed (same engine)
                vector.wait_ge(sem_act, 2 * t + 1)
                nc.vector.scalar_tensor_tensor(
                    sb_Vt[:], gs[:, 32:64], 1.0, sb_C[:, t * 32:(t + 1) * 32],
                    Alu.add, Alu.mult)
                nc.vector.scalar_tensor_tensor(
                    sb_U[:], gs[:, 0:32], 1.0, gs[:, 96:128],
                    Alu.add, Alu.mult).then_inc(sem_uv)
                vector.wait_ge(sem_uv, t + 1)  # u/v committed (in-order pipe)
                nc.vector.tensor_add(ss, sb_U[:], sb_Vt[:]).then_inc(sem_cp)
                # true cell state for next step; hides under ACT tanh_c
                vector.wait_ge(sem_cp, t + 1)
                nc.vector.tensor_scalar_mul(
                    sb_C[:, (t + 1) * 32:(t + 2) * 32], ss, 0.5
                ).then_inc(sem_cv)
                # h2 = (to+1)(.)tanh(c') = 2h; h-consumers use halved weights
                vector.wait_ge(sem_act, 2 * t + 2)
                nc.vector.scalar_tensor_tensor(
                    sb_H[:, (t + 1) * 32:(t + 2) * 32], gs[:, 64:96], 1.0,
                    sb_TC[:, s * 32:(s + 1) * 32], Alu.add, Alu.mult
                ).then_inc(sem_h)
                if t < M_TILES:
                    pre_copy(M_TILES + t)
            # ---- head copies ----
            vector.wait_ge(dma_bo, 16)
            for half in range(n_half):
                vector.wait_ge(sem_hd, K2 * (half + 1))
                nc.vector.tensor_scalar_add(
                    sb_head[:, half * HALF:(half + 1) * HALF],
                    ps_hd[half][0:11, 0:HALF], sb_bob[0:11, 0:1],
                ).then_inc(sem_hdcp)

        @block.scalar
        def _(scalar):
            Tanh = mybir.ActivationFunctionType.Tanh
            for t in range(T):
                s = t % 2
                gs = sb_G[:, s * 128:(s + 1) * 128]
                scalar.wait_ge(sem_pe, t + 1)
                scalar.activation(gs[:, 0:128], ps_z[s][:, 0:128], Tanh
                                  ).then_inc(sem_act)
                scalar.wait_ge(sem_cp, t + 1)
                scalar.activation(sb_TC[:, s * 32:(s + 1) * 32],
                                  sb_S[:, s * 32:(s + 1) * 32], Tanh,
                                  scale=0.5).then_inc(sem_act)

    return nc, ctx


_BUILD_CACHE = {}


def _get_nc(T):
    if T not in _BUILD_CACHE:
        _BUILD_CACHE[T] = _build(T)
    return _BUILD_CACHE[T][0]


def _prep_inputs(X, u, Wk, Wr, b_lstm, Wo, bo, Wc, bc, T):
    """Build the 8 per-core input maps (numpy, host-side sharding)."""
    # column scaling: i,f,o gates get 0.5 (sigma(x) = (tanh(x/2)+1)/2);
    # row scaling: recurrent/head weights get 0.5 because h is stored as 2h.
    col_scale = np.ones((1, 1024), np.float32)
    col_scale[:, :768] = 0.5          # i, f, o blocks after GATE_PERM
    Wk_p = np.ascontiguousarray(Wk[:, GATE_PERM] * col_scale
                                ).astype(np.float16)
    Wr_p = (Wr[:, GATE_PERM].astype(np.float32) * col_scale) * 0.5
    WrT = np.ascontiguousarray(
        Wr_p.reshape(2, 128, 1024).transpose(1, 0, 2).reshape(128, 2048)
    ).astype(np.float16)
    blstm = np.ascontiguousarray(
        (b_lstm[GATE_PERM].astype(np.float32) * col_scale[0]
         ).reshape(8, 128).T)
    WoC = np.concatenate([Wo.astype(np.float32),
                          Wc[:256].astype(np.float32)], axis=1) * 0.5
    WoC = np.ascontiguousarray(
        WoC.reshape(2, 128, 11).transpose(1, 0, 2).reshape(128, 22)
    ).astype(np.float16)
    bob = np.concatenate([bo.astype(np.float32), [0.0]]).reshape(11, 1)
    bob = np.ascontiguousarray(bob, np.float32)

    ident = np.eye(128, dtype=np.float16)
    in_maps = []
    for i in range(NCORES):
        bsl = slice(i * BL, (i + 1) * BL)
        Xt = np.ascontiguousarray(
            X[bsl, :T, :].astype(np.float32).transpose(2, 1, 0)
            .reshape(128, T * BL)).astype(np.float16)
        in_maps.append({
            "Xt": Xt, "WkT": Wk_p, "WrT": WrT, "blstm": blstm,
            "WoC": WoC, "bob": bob, "ident": ident,
        })
    return in_maps


def _sigmoid64(x):
    return 1.0 / (1.0 + np.exp(-x.astype(np.float64)))


def _softmax32(x):
    x = x.astype(np.float32)
    e = np.exp(x - x.max(axis=-1, keepdims=True))
    return (e / e.sum(axis=-1, keepdims=True)).astype(np.float32)


def _fallback_scan(x_seq, u_seq, h0, c0, t0, Wk, Wr, b_lstm, Wo, bo, Wc, bc):
    """Continue the reference recurrence on host for one sample that did not
    halt by t0.  Returns the sample's output row (float32)."""
    h = h0.astype(np.float32).copy()
    c = c0.astype(np.float32).copy()
    Wk = Wk.astype(np.float32); Wr = Wr.astype(np.float32)
    b_lstm = b_lstm.astype(np.float32)
    sig = lambda v: 1.0 / (1.0 + np.exp(-v))
    Tt = x_seq.shape[0]
    logits_last = None
    for t in range(t0, Tt):
        z = x_seq[t] @ Wk + h @ Wr + b_lstm
        i, f, g, o = np.split(z, 4)
        i = sig(i); f = sig(f); g = np.tanh(g); o = sig(o)
        c = f * c + i * g
        h = o * np.tanh(c)
        y = h @ Wo.astype(np.float32) + bo.astype(np.float32)
        logits = _softmax32(y)
        pre = float(h @ Wc[:256, 0].astype(np.float32)) \
            + t * float(Wc[256, 0]) + float(bc[0])
        probs = (1.0 - EPS) * sig(np.float32(pre)) + EPS * 0.05
        if u_seq[t] < probs:
            return logits
        logits_last = logits
    return logits_last


def kernel(**inputs):
    X = np.asarray(inputs["X"], np.float32)
    u = np.asarray(inputs["u"], np.float32)
    Wk = np.asarray(inputs["Wk"], np.float32)
    Wr = np.asarray(inputs["Wr"], np.float32)
    b_lstm = np.asarray(inputs["b_lstm"], np.float32)
    Wo = np.asarray(inputs["Wo"], np.float32)
    bo = np.asarray(inputs["bo"], np.float32)
    Wc = np.asarray(inputs["Wc"], np.float32)
    bc = np.asarray(inputs["bc"], np.float32)
    T = T_EFF

    nc = _get_nc(T)
    in_maps = _prep_inputs(X, u, Wk, Wr, b_lstm, Wo, bo, Wc, bc, T)
    res = run_bass_kernel_spmd(nc, in_maps, list(range(NCORES)))

    wc_t = float(Wc[256, 0])
    bias_c = float(bc[0])
    tvec = np.arange(T, dtype=np.float64)

    out = np.zeros((B, C), np.float32)
    for i in range(NCORES):
        bsl = slice(i * BL, (i + 1) * BL)
        head = res.results[i]["head"]          # [11, T*BL]
        y_pre = head[0:10].reshape(10, T, BL).transpose(1, 2, 0)  # [T, b, 10]
        pre_c = head[10].reshape(T, BL).astype(np.float64)        # [T, b]
        probs = (1.0 - EPS) * _sigmoid64(pre_c + tvec[:, None] * wc_t + bias_c) \
            + EPS * 0.05
        u_core = u[bsl, :T, 0]                 # [b, T]
        a = u_core.T.astype(np.float64) < probs  # [T, b]
        halted = a.any(axis=0)
        tstar = np.argmax(a, axis=0)           # first halt step per sample
        logits = _softmax32(y_pre)             # [T, b, 10]
        for b_ in range(BL):
            if halted[b_]:
                out[i * BL + b_] = logits[tstar[b_], b_]
            else:
                sh = res.results[i]["state_h"].astype(np.float32) * 0.5
                sc = res.results[i]["state_c"].astype(np.float32)
                h_T = sh.reshape(128, 2, BL).transpose(2, 1, 0) \
                    .reshape(BL, 256)[b_]
                c_T = sc.reshape(128, 2, BL).transpose(2, 1, 0) \
                    .reshape(BL, 256)[b_]
                out[i * BL + b_] = _fallback_scan(
                    X[i * BL + b_], u[i * BL + b_, :, 0], h_T, c_T, T,
                    Wk, Wr, b_lstm, Wo, bo, Wc, bc)
    return out


# revision 15
# speedup vs baseline: 4.2850x; 1.0762x over previous
# Trainium2 Bass kernel for nn_EARLIEST (adaptive-halting LSTM, B=128 T=4096
# V=128 H=256 C=10).
#
# Key observation: the model halts each batch sample at the first step t where
# u[b,t] < probs[b,t], with probs ~= 0.45 early on, so every sample halts
# within a few dozen steps (max 36 for the seed-0 inputs).  The returned
# output only needs logits at each sample's first halt step (or step T-1 for
# never-halted samples).  So the device kernel runs the LSTM scan for only
# T_EFF timesteps, emits pre-softmax logits and the halting dot-product for
# every (t, b), and the host applies the (exact) halting latch.  A numpy
# fallback continues the recurrence from the device's (h, c) state for any
# sample that has not halted by T_EFF (statistically never happens; the
# fallback keeps the kernel correct for arbitrary inputs).
#
# Sharding: data-parallel over batch, 16 samples per core, weights replicated.
# Layout on device is feature-major: h^T is [H=256, b=16] stored as two
# 128-partition k-tiles side by side, so LSTM gate math runs on full
# 128-partition tiles and the recurrent matmuls need no transposes.

import numpy as np
import ml_dtypes

import concourse.bass as bass
import concourse.mybir as mybir
from concourse.bass_utils import run_bass_kernel_spmd

B, T_FULL, V, H, C = 128, 4096, 128, 256, 10
EPS = 0.1
NCORES = 8
BL = B // NCORES  # 16 samples per core
T_EFF = 48
M_TILES = 8   # 4H/128
K2 = 2        # H/128
F32 = mybir.dt.float32
F16 = mybir.dt.float16

# gate order in z is (i, f, g, o); we permute weight columns to (i, f, o, g)
# so one sigmoid covers a contiguous [i|f|o] block and one tanh covers g.
GATE_PERM = np.concatenate([np.arange(0, 512), np.arange(768, 1024),
                            np.arange(512, 768)])


def _build(T):
    """Build the raw-bass single-core program (SPMD across 8 cores)."""
    nc = bass.Bass()

    d_Xt = nc.dram_tensor("Xt", [128, T * BL], F16, kind="ExternalInput")
    d_WkT = nc.dram_tensor("WkT", [128, 1024], F16, kind="ExternalInput")
    d_WrT = nc.dram_tensor("WrT", [128, 2048], F16, kind="ExternalInput")
    d_ident = nc.dram_tensor("ident", [128, 128], F16, kind="ExternalInput")
    d_blstm = nc.dram_tensor("blstm", [128, 8], F32, kind="ExternalInput")
    d_WoC = nc.dram_tensor("WoC", [128, 22], F16, kind="ExternalInput")
    d_bob = nc.dram_tensor("bob", [11, 1], F32, kind="ExternalInput")
    d_head = nc.dram_tensor("head", [11, T * BL], F32, kind="ExternalOutput")
    d_state_h = nc.dram_tensor("state_h", [128, 32], F16, kind="ExternalOutput")
    d_state_c = nc.dram_tensor("state_c", [128, 32], F32, kind="ExternalOutput")

    NH = T * BL
    HALF = NH // 2  # fp32 head matmul moving-operand limit is 512

    from contextlib import ExitStack
    ctx = ExitStack()
    sb_Xt = ctx.enter_context(nc.sbuf_tensor([128, T * BL], F16))
    sb_WkT = ctx.enter_context(nc.sbuf_tensor([128, 1024], F16))
    sb_WrT = ctx.enter_context(nc.sbuf_tensor([128, 2048], F16))
    sb_I = ctx.enter_context(nc.sbuf_tensor([128, 128], F16))
    sb_blstm = ctx.enter_context(nc.sbuf_tensor([128, 8], F32))
    sb_WoC = ctx.enter_context(nc.sbuf_tensor([128, 22], F16))
    sb_bob = ctx.enter_context(nc.sbuf_tensor([11, 1], F32))
    sb_XW = ctx.enter_context(nc.sbuf_tensor([128, T * 128], F16))
    sb_H = ctx.enter_context(nc.sbuf_tensor([128, (T + 1) * 32], F16))
    sb_C = ctx.enter_context(nc.sbuf_tensor([128, (T + 1) * 32], F32))
    sb_G = ctx.enter_context(nc.sbuf_tensor([128, 2 * 128], F32))
    sb_TC = ctx.enter_context(nc.sbuf_tensor([128, 2 * 32], F32))
    sb_S = ctx.enter_context(nc.sbuf_tensor([128, 2 * 32], F32))
    sb_U = ctx.enter_context(nc.sbuf_tensor([128, 32], F32))
    sb_Vt = ctx.enter_context(nc.sbuf_tensor([128, 32], F32))
    sb_head = ctx.enter_context(nc.sbuf_tensor([11, T * BL], F32))

    ps_pre = [ctx.enter_context(nc.psum_tensor(f"ps_pre{j}", [128, 512], F32))
              for j in range(2)]
    ps_z = [ctx.enter_context(nc.psum_tensor(f"ps_z{j}", [128, 512], F32))
            for j in range(2)]
    ps_hd = [ctx.enter_context(nc.psum_tensor(f"ps_hd{j}", [128, 512], F32))
             for j in range(2)]
    ps_s = ctx.enter_context(nc.psum_tensor("ps_s", [128, 512], F32))

    # one semaphore per input load: DMA completion order is not program order
    dma_xt = ctx.enter_context(nc.semaphore("dma_xt"))
    dma_wk = ctx.enter_context(nc.semaphore("dma_wk"))
    dma_wr = ctx.enter_context(nc.semaphore("dma_wr"))
    dma_id = ctx.enter_context(nc.semaphore("dma_id"))
    dma_bl = ctx.enter_context(nc.semaphore("dma_bl"))
    dma_wo = ctx.enter_context(nc.semaphore("dma_wo"))
    dma_bo = ctx.enter_context(nc.semaphore("dma_bo"))
    dma_out = ctx.enter_context(nc.semaphore("dma_out"))
    sem_pre = ctx.enter_context(nc.semaphore("sem_pre"))
    sem_precp = ctx.enter_context(nc.semaphore("sem_precp"))
    sem_h = ctx.enter_context(nc.semaphore("sem_h"))
    sem_cp = ctx.enter_context(nc.semaphore("sem_cp"))
    sem_act = ctx.enter_context(nc.semaphore("sem_act"))
    sem_pe = ctx.enter_context(nc.semaphore("sem_pe"))
    sem_hd = ctx.enter_context(nc.semaphore("sem_hd"))
    sem_hdcp = ctx.enter_context(nc.semaphore("sem_hdcp"))
    sem_uv = ctx.enter_context(nc.semaphore("sem_uv"))
    sem_cv = ctx.enter_context(nc.semaphore("sem_cv"))

    n_half = 2
    assert T % n_half == 0
    TH = T // n_half
    assert TH * BL == HALF

    with nc.Block() as block:

        @block.sync
        def _(sync):
            sync.dma_start(out=sb_Xt[:], in_=d_Xt[:]).then_inc(dma_xt, 16)
            sync.dma_start(out=sb_WkT[:], in_=d_WkT[:]).then_inc(dma_wk, 16)
            sync.dma_start(out=sb_WrT[:], in_=d_WrT[:]).then_inc(dma_wr, 16)
            sync.dma_start(out=sb_I[:], in_=d_ident[:]).then_inc(dma_id, 16)
            sync.dma_start(out=sb_blstm[:], in_=d_blstm[:]).then_inc(dma_bl, 16)
            sync.dma_start(out=sb_WoC[:], in_=d_WoC[:]).then_inc(dma_wo, 16)
            sync.dma_start(out=sb_bob[:], in_=d_bob[:]).then_inc(dma_bo, 16)
            sync.wait_ge(sem_hdcp, n_half)
            sync.dma_start(out=d_head[:], in_=sb_head[:]).then_inc(dma_out, 16)
            sync.wait_ge(sem_h, T + 1)
            sync.dma_start(out=d_state_h[:],
                           in_=sb_H[:, T * 32:(T + 1) * 32]).then_inc(dma_out, 16)
            sync.wait_ge(sem_cv, T)
            sync.dma_start(out=d_state_c[:],
                           in_=sb_C[:, T * 32:(T + 1) * 32]).then_inc(dma_out, 16)
            sync.wait_ge(dma_out, 48)

        @block.tensor
        def _(tensor):
            # ---- precompute XW = Wk^T X^T (feature-major, fp16) ----
            # half 0 runs up front; half 1 is interleaved into the scan.
            def pre_mm(idx):
                half, m = divmod(idx, M_TILES)
                if idx >= 2:
                    tensor.wait_ge(sem_precp, idx - 1)
                tensor.matmul(
                    ps_pre[idx % 2][:, 0:HALF],
                    sb_WkT[:, m * 128:(m + 1) * 128],
                    sb_Xt[:, half * HALF:(half + 1) * HALF],
                    start=True, stop=True,
                ).then_inc(sem_pre)

            tensor.wait_ge(dma_xt, 16)
            tensor.wait_ge(dma_wk, 16)
            for idx in range(M_TILES):
                pre_mm(idx)
            # ---- recurrent scan ----
            tensor.wait_ge(dma_wr, 16)
            tensor.wait_ge(dma_id, 16)
            tensor.wait_ge(sem_precp, M_TILES)   # XW half 0 in SBUF
            for t in range(T):
                if t == TH:
                    tensor.wait_ge(sem_precp, 2 * M_TILES)  # XW half 1
                if t >= 2:
                    # ps_z bank reuse: ACT consumed step t-2 gates
                    tensor.wait_ge(sem_act, 2 * (t - 2) + 1)
                # prefill z with XW[t] in one identity matmul BEFORE waiting
                # for h: it only depends on XW, so it runs during the tail
                # of step t-1 (and keeps PE a little warmer).
                tensor.matmul(ps_z[t % 2][:, 0:128], sb_I[:],
                              sb_XW[:, t * 128:(t + 1) * 128],
                              start=True, stop=True, skip_group_check=True)
                tensor.wait_ge(sem_h, t + 1)
                for m in range(M_TILES):
                    for k in range(K2):
                        mm = tensor.matmul(
                            ps_z[t % 2][:, m * BL:(m + 1) * BL],
                            sb_WrT[:, k * 1024 + m * 128:k * 1024 + (m + 1) * 128],
                            sb_H[:, t * 32 + k * BL:t * 32 + (k + 1) * BL],
                            start=False, stop=False, skip_group_check=True,
                        )
                mm.then_inc(sem_pe)
                if t < M_TILES:
                    pre_mm(M_TILES + t)
            # ---- head: [Wo | Wc]^T applied to all h_t ----
            tensor.wait_ge(dma_wo, 16)
            tensor.wait_ge(sem_h, T + 1)
            h4 = sb_H[:].rearrange("p (t k b) -> p t k b", k=K2, b=BL)
            for half in range(n_half):
                for k in range(K2):
                    tensor.matmul(
                        ps_hd[half][0:11, 0:HALF],
                        sb_WoC[:, k * 11:(k + 1) * 11],
                        h4[:, 1 + half * TH:1 + (half + 1) * TH, k, :],
                        start=(k == 0), stop=(k == 1),
                    ).then_inc(sem_hd)

        @block.vector
        def _(vector):
            vector.memset(sb_H[:, 0:32], 0.0)
            vector.memset(sb_C[:, 0:32], 0.0).then_inc(sem_h)
            # ---- precompute copies: psum + b_lstm -> XW (fp16) ----
            vector.wait_ge(dma_bl, 16)
            xw4 = sb_XW[:].rearrange("p (t m b) -> p t m b", m=M_TILES, b=BL)

            def pre_copy(idx):
                half, m = divmod(idx, M_TILES)
                vector.wait_ge(sem_pre, idx + 1)
                psrc = ps_pre[idx % 2][:, 0:HALF].rearrange(
                    "p (t b) -> p t b", b=BL)
                nc.vector.tensor_scalar_add(
                    xw4[:, half * TH:(half + 1) * TH, m, :], psrc,
                    sb_blstm[:, m:m + 1],
                ).then_inc(sem_precp)

            for idx in range(M_TILES):
                pre_copy(idx)
            # same-engine fence: v2(0) reads C written by memset above
            vector.drain()
            # ---- scan pointwise ----
            Alu = mybir.AluOpType
            for t in range(T):
                s = t % 2
                gs = sb_G[:, s * 128:(s + 1) * 128]
                ss = ps_s[:, s * 32:(s + 1) * 32]
                # all gates arrive as tanh (i,f,o weight cols pre-halved on
                # host): v2 = (tf+1)(.)c = 2f(.)c ; u2 = (ti+1)(.)tg ; S = 2c'
                if t >= 1:
                    vector.wait_ge(sem_cv, t)  # c(t) committed (same engine)
                vector.wait_ge(sem_act, 2 * t + 1)
                nc.vector.scalar_tensor_tensor(
                    sb_Vt[:], gs[:, 32:64], 1.0, sb_C[:, t * 32:(t + 1) * 32],
                    Alu.add, Alu.mult)
                nc.vector.scalar_tensor_tensor(
                    sb_U[:], gs[:, 0:32], 1.0, gs[:, 96:128],
                    Alu.add, Alu.mult).then_inc(sem_uv)
                vector.wait_ge(sem_uv, t + 1)  # u/v committed (in-order pipe)
                nc.vector.tensor_add(ss, sb_U[:], sb_Vt[:]).then_inc(sem_cp)
                # true cell state for next step; hides under ACT tanh_c
                vector.wait_ge(sem_cp, t + 1)
                nc.vector.tensor_scalar_mul(
                    sb_C[:, (t + 1) * 32:(t + 2) * 32], ss, 0.5
                ).then_inc(sem_cv)
                # h2 = (to+1)(.)tanh(c') = 2h; h-consumers use halved weights
                vector.wait_ge(sem_act, 2 * t + 2)
                nc.vector.scalar_tensor_tensor(
                    sb_H[:, (t + 1) * 32:(t + 2) * 32], gs[:, 64:96], 1.0,
                    sb_TC[:, s * 32:(s + 1) * 32], Alu.add, Alu.mult
                ).then_inc(sem_h)
                if t < M_TILES:
                    pre_copy(M_TILES + t)
            # ---- head copies ----
            vector.wait_ge(dma_bo, 16)
            for half in range(n_half):
                vector.wait_ge(sem_hd, K2 * (half + 1))
                nc.vector.tensor_scalar_add(
                    sb_head[:, half * HALF:(half + 1) * HALF],
                    ps_hd[half][0:11, 0:HALF], sb_bob[0:11, 0:1],
                ).then_inc(sem_hdcp)

        @block.scalar
        def _(scalar):
            Tanh = mybir.ActivationFunctionType.Tanh
            for t in range(T):
                s = t % 2
                gs = sb_G[:, s * 128:(s + 1) * 128]
                scalar.wait_ge(sem_pe, t + 1)
                scalar.activation(gs[:, 0:128], ps_z[s][:, 0:128], Tanh
                                  ).then_inc(sem_act)
                scalar.wait_ge(sem_cp, t + 1)
                scalar.activation(sb_TC[:, s * 32:(s + 1) * 32],
                                  ps_s[:, s * 32:(s + 1) * 32], Tanh,
                                  scale=0.5).then_inc(sem_act)

    return nc, ctx


_BUILD_CACHE = {}


def _get_nc(T):
    if T not in _BUILD_CACHE:
        _BUILD_CACHE[T] = _build(T)
    return _BUILD_CACHE[T][0]


def _prep_inputs(X, u, Wk, Wr, b_lstm, Wo, bo, Wc, bc, T):
    """Build the 8 per-core input maps (numpy, host-side sharding)."""
    # column scaling: i,f,o gates get 0.5 (sigma(x) = (tanh(x/2)+1)/2);
    # row scaling: recurrent/head weights get 0.5 because h is stored as 2h.
    col_scale = np.ones((1, 1024), np.float32)
    col_scale[:, :768] = 0.5          # i, f, o blocks after GATE_PERM
    Wk_p = np.ascontiguousarray(Wk[:, GATE_PERM] * col_scale
                                ).astype(np.float16)
    Wr_p = (Wr[:, GATE_PERM].astype(np.float32) * col_scale) * 0.5
    WrT = np.ascontiguousarray(
        Wr_p.reshape(2, 128, 1024).transpose(1, 0, 2).reshape(128, 2048)
    ).astype(np.float16)
    blstm = np.ascontiguousarray(
        (b_lstm[GATE_PERM].astype(np.float32) * col_scale[0]
         ).reshape(8, 128).T)
    WoC = np.concatenate([Wo.astype(np.float32),
                          Wc[:256].astype(np.float32)], axis=1) * 0.5
    WoC = np.ascontiguousarray(
        WoC.reshape(2, 128, 11).transpose(1, 0, 2).reshape(128, 22)
    ).astype(np.float16)
    bob = np.concatenate([bo.astype(np.float32), [0.0]]).reshape(11, 1)
    bob = np.ascontiguousarray(bob, np.float32)

    ident = np.eye(128, dtype=np.float16)
    in_maps = []
    for i in range(NCORES):
        bsl = slice(i * BL, (i + 1) * BL)
        Xt = np.ascontiguousarray(
            X[bsl, :T, :].astype(np.float32).transpose(2, 1, 0)
            .reshape(128, T * BL)).astype(np.float16)
        in_maps.append({
            "Xt": Xt, "WkT": Wk_p, "WrT": WrT, "blstm": blstm,
            "WoC": WoC, "bob": bob, "ident": ident,
        })
    return in_maps


def _sigmoid64(x):
    return 1.0 / (1.0 + np.exp(-x.astype(np.float64)))


def _softmax32(x):
    x = x.astype(np.float32)
    e = np.exp(x - x.max(axis=-1, keepdims=True))
    return (e / e.sum(axis=-1, keepdims=True)).astype(np.float32)


def _fallback_scan(x_seq, u_seq, h0, c0, t0, Wk, Wr, b_lstm, Wo, bo, Wc, bc):
    """Continue the reference recurrence on host for one sample that did not
    halt by t0.  Returns the sample's output row (float32)."""
    h = h0.astype(np.float32).copy()
    c = c0.astype(np.float32).copy()
    Wk = Wk.astype(np.float32); Wr = Wr.astype(np.float32)
    b_lstm = b_lstm.astype(np.float32)
    sig = lambda v: 1.0 / (1.0 + np.exp(-v))
    Tt = x_seq.shape[0]
    logits_last = None
    for t in range(t0, Tt):
        z = x_seq[t] @ Wk + h @ Wr + b_lstm
        i, f, g, o = np.split(z, 4)
        i = sig(i); f = sig(f); g = np.tanh(g); o = sig(o)
        c = f * c + i * g
        h = o * np.tanh(c)
        y = h @ Wo.astype(np.float32) + bo.astype(np.float32)
        logits = _softmax32(y)
        pre = float(h @ Wc[:256, 0].astype(np.float32)) \
            + t * float(Wc[256, 0]) + float(bc[0])
        probs = (1.0 - EPS) * sig(np.float32(pre)) + EPS * 0.05
        if u_seq[t] < probs:
            return logits
        logits_last = logits
    return logits_last


def kernel(**inputs):
    X = np.asarray(inputs["X"], np.float32)
    u = np.asarray(inputs["u"], np.float32)
    Wk = np.asarray(inputs["Wk"], np.float32)
    Wr = np.asarray(inputs["Wr"], np.float32)
    b_lstm = np.asarray(inputs["b_lstm"], np.float32)
    Wo = np.asarray(inputs["Wo"], np.float32)
    bo = np.asarray(inputs["bo"], np.float32)
    Wc = np.asarray(inputs["Wc"], np.float32)
    bc = np.asarray(inputs["bc"], np.float32)
    T = T_EFF

    nc = _get_nc(T)
    in_maps = _prep_inputs(X, u, Wk, Wr, b_lstm, Wo, bo, Wc, bc, T)
    res = run_bass_kernel_spmd(nc, in_maps, list(range(NCORES)))

    wc_t = float(Wc[256, 0])
    bias_c = float(bc[0])
    tvec = np.arange(T, dtype=np.float64)

    out = np.zeros((B, C), np.float32)
    for i in range(NCORES):
        bsl = slice(i * BL, (i + 1) * BL)
        head = res.results[i]["head"]          # [11, T*BL]
        y_pre = head[0:10].reshape(10, T, BL).transpose(1, 2, 0)  # [T, b, 10]
        pre_c = head[10].reshape(T, BL).astype(np.float64)        # [T, b]
        probs = (1.0 - EPS) * _sigmoid64(pre_c + tvec[:, None] * wc_t + bias_c) \
            + EPS * 0.05
        u_core = u[bsl, :T, 0]                 # [b, T]
        a = u_core.T.astype(np.float64) < probs  # [T, b]
        halted = a.any(axis=0)
        tstar = np.argmax(a, axis=0)           # first halt step per sample
        logits = _softmax32(y_pre)             # [T, b, 10]
        for b_ in range(BL):
            if halted[b_]:
                out[i * BL + b_] = logits[tstar[b_], b_]
            else:
                sh = res.results[i]["state_h"].astype(np.float32) * 0.5
                sc = res.results[i]["state_c"].astype(np.float32)
                h_T = sh.reshape(128, 2, BL).transpose(2, 1, 0) \
                    .reshape(BL, 256)[b_]
                c_T = sc.reshape(128, 2, BL).transpose(2, 1, 0) \
                    .reshape(BL, 256)[b_]
                out[i * BL + b_] = _fallback_scan(
                    X[i * BL + b_], u[i * BL + b_, :, 0], h_T, c_T, T,
                    Wk, Wr, b_lstm, Wo, bo, Wc, bc)
    return out


# revision 16
# speedup vs baseline: 4.4581x; 1.0404x over previous
# Trainium2 Bass kernel for nn_EARLIEST (adaptive-halting LSTM, B=128 T=4096
# V=128 H=256 C=10).
#
# Key observation: the model halts each batch sample at the first step t where
# u[b,t] < probs[b,t], with probs ~= 0.45 early on, so every sample halts
# within a few dozen steps (max 36 for the seed-0 inputs).  The returned
# output only needs logits at each sample's first halt step (or step T-1 for
# never-halted samples).  So the device kernel runs the LSTM scan for only
# T_EFF timesteps, emits pre-softmax logits and the halting dot-product for
# every (t, b), and the host applies the (exact) halting latch.  A numpy
# fallback continues the recurrence from the device's (h, c) state for any
# sample that has not halted by T_EFF (statistically never happens; the
# fallback keeps the kernel correct for arbitrary inputs).
#
# Sharding: data-parallel over batch, 16 samples per core, weights replicated.
# Layout on device is feature-major: h^T is [H=256, b=16] stored as two
# 128-partition k-tiles side by side, so LSTM gate math runs on full
# 128-partition tiles and the recurrent matmuls need no transposes.

import numpy as np
import ml_dtypes

import concourse.bass as bass
import concourse.mybir as mybir
from concourse.bass_utils import run_bass_kernel_spmd

B, T_FULL, V, H, C = 128, 4096, 128, 256, 10
EPS = 0.1
NCORES = 8
BL = B // NCORES  # 16 samples per core
T_EFF = 48
M_TILES = 8   # 4H/128
K2 = 2        # H/128
F32 = mybir.dt.float32
F16 = mybir.dt.float16

# gate order stays the native (i, f, g, o): with the all-tanh trick the
# only contiguity needed is [i,f,g] (first ACT chunk) and [o] (second).
GATE_PERM = np.arange(1024)


def _build(T):
    """Build the raw-bass single-core program (SPMD across 8 cores)."""
    nc = bass.Bass()

    d_Xt = nc.dram_tensor("Xt", [128, T * BL], F16, kind="ExternalInput")
    d_WkT = nc.dram_tensor("WkT", [128, 1024], F16, kind="ExternalInput")
    d_WrT = nc.dram_tensor("WrT", [128, 2048], F16, kind="ExternalInput")
    d_ident = nc.dram_tensor("ident", [128, 128], F16, kind="ExternalInput")
    d_blstm = nc.dram_tensor("blstm", [128, 8], F32, kind="ExternalInput")
    d_WoC = nc.dram_tensor("WoC", [128, 22], F16, kind="ExternalInput")
    d_bob = nc.dram_tensor("bob", [11, 1], F32, kind="ExternalInput")
    d_head = nc.dram_tensor("head", [11, T * BL], F32, kind="ExternalOutput")
    d_state_h = nc.dram_tensor("state_h", [128, 32], F16, kind="ExternalOutput")
    d_state_c = nc.dram_tensor("state_c", [128, 32], F32, kind="ExternalOutput")

    NH = T * BL
    HALF = NH // 2  # fp32 head matmul moving-operand limit is 512

    from contextlib import ExitStack
    ctx = ExitStack()
    sb_Xt = ctx.enter_context(nc.sbuf_tensor([128, T * BL], F16))
    sb_WkT = ctx.enter_context(nc.sbuf_tensor([128, 1024], F16))
    sb_WrT = ctx.enter_context(nc.sbuf_tensor([128, 2048], F16))
    sb_I = ctx.enter_context(nc.sbuf_tensor([128, 128], F16))
    sb_blstm = ctx.enter_context(nc.sbuf_tensor([128, 8], F32))
    sb_WoC = ctx.enter_context(nc.sbuf_tensor([128, 22], F16))
    sb_bob = ctx.enter_context(nc.sbuf_tensor([11, 1], F32))
    sb_XW = ctx.enter_context(nc.sbuf_tensor([128, T * 128], F16))
    sb_H = ctx.enter_context(nc.sbuf_tensor([128, (T + 1) * 32], F16))
    sb_C = ctx.enter_context(nc.sbuf_tensor([128, (T + 1) * 32], F32))
    sb_G = ctx.enter_context(nc.sbuf_tensor([128, 2 * 128], F32))
    sb_TC = ctx.enter_context(nc.sbuf_tensor([128, 2 * 32], F32))
    sb_S = ctx.enter_context(nc.sbuf_tensor([128, 2 * 32], F32))
    sb_U = ctx.enter_context(nc.sbuf_tensor([128, 32], F32))
    sb_Vt = ctx.enter_context(nc.sbuf_tensor([128, 32], F32))
    sb_head = ctx.enter_context(nc.sbuf_tensor([11, T * BL], F32))

    ps_pre = [ctx.enter_context(nc.psum_tensor(f"ps_pre{j}", [128, 512], F32))
              for j in range(2)]
    ps_z = [ctx.enter_context(nc.psum_tensor(f"ps_z{j}", [128, 512], F32))
            for j in range(2)]
    ps_hd = [ctx.enter_context(nc.psum_tensor(f"ps_hd{j}", [128, 512], F32))
             for j in range(2)]
    ps_s = ctx.enter_context(nc.psum_tensor("ps_s", [128, 512], F32))

    # one semaphore per input load: DMA completion order is not program order
    dma_xt = ctx.enter_context(nc.semaphore("dma_xt"))
    dma_wk = ctx.enter_context(nc.semaphore("dma_wk"))
    dma_wr = ctx.enter_context(nc.semaphore("dma_wr"))
    dma_id = ctx.enter_context(nc.semaphore("dma_id"))
    dma_bl = ctx.enter_context(nc.semaphore("dma_bl"))
    dma_wo = ctx.enter_context(nc.semaphore("dma_wo"))
    dma_bo = ctx.enter_context(nc.semaphore("dma_bo"))
    dma_out = ctx.enter_context(nc.semaphore("dma_out"))
    sem_pre = ctx.enter_context(nc.semaphore("sem_pre"))
    sem_precp = ctx.enter_context(nc.semaphore("sem_precp"))
    sem_h = ctx.enter_context(nc.semaphore("sem_h"))
    sem_cp = ctx.enter_context(nc.semaphore("sem_cp"))
    sem_act = ctx.enter_context(nc.semaphore("sem_act"))
    sem_pe = ctx.enter_context(nc.semaphore("sem_pe"))
    sem_hd = ctx.enter_context(nc.semaphore("sem_hd"))
    sem_hdcp = ctx.enter_context(nc.semaphore("sem_hdcp"))
    sem_uv = ctx.enter_context(nc.semaphore("sem_uv"))
    sem_cv = ctx.enter_context(nc.semaphore("sem_cv"))

    n_half = 2
    assert T % n_half == 0
    TH = T // n_half
    assert TH * BL == HALF

    with nc.Block() as block:

        @block.sync
        def _(sync):
            sync.dma_start(out=sb_Xt[:], in_=d_Xt[:]).then_inc(dma_xt, 16)
            sync.dma_start(out=sb_WkT[:], in_=d_WkT[:]).then_inc(dma_wk, 16)
            sync.dma_start(out=sb_WrT[:], in_=d_WrT[:]).then_inc(dma_wr, 16)
            sync.dma_start(out=sb_I[:], in_=d_ident[:]).then_inc(dma_id, 16)
            sync.dma_start(out=sb_blstm[:], in_=d_blstm[:]).then_inc(dma_bl, 16)
            sync.dma_start(out=sb_WoC[:], in_=d_WoC[:]).then_inc(dma_wo, 16)
            sync.dma_start(out=sb_bob[:], in_=d_bob[:]).then_inc(dma_bo, 16)
            sync.wait_ge(sem_hdcp, n_half)
            sync.dma_start(out=d_head[:], in_=sb_head[:]).then_inc(dma_out, 16)
            sync.wait_ge(sem_h, T + 1)
            sync.dma_start(out=d_state_h[:],
                           in_=sb_H[:, T * 32:(T + 1) * 32]).then_inc(dma_out, 16)
            sync.wait_ge(sem_cv, T)
            sync.dma_start(out=d_state_c[:],
                           in_=sb_C[:, T * 32:(T + 1) * 32]).then_inc(dma_out, 16)
            sync.wait_ge(dma_out, 48)

        @block.tensor
        def _(tensor):
            # ---- precompute XW = Wk^T X^T (feature-major, fp16) ----
            # half 0 runs up front; half 1 is interleaved into the scan.
            def pre_mm(idx):
                half, m = divmod(idx, M_TILES)
                if idx >= 2:
                    tensor.wait_ge(sem_precp, idx - 1)
                tensor.matmul(
                    ps_pre[idx % 2][:, 0:HALF],
                    sb_WkT[:, m * 128:(m + 1) * 128],
                    sb_Xt[:, half * HALF:(half + 1) * HALF],
                    start=True, stop=True,
                ).then_inc(sem_pre)

            tensor.wait_ge(dma_xt, 16)
            tensor.wait_ge(dma_wk, 16)
            for idx in range(M_TILES):
                pre_mm(idx)
            # ---- recurrent scan ----
            tensor.wait_ge(dma_wr, 16)
            tensor.wait_ge(dma_id, 16)
            tensor.wait_ge(dma_wo, 16)
            tensor.wait_ge(sem_precp, M_TILES)   # XW half 0 in SBUF
            h4 = sb_H[:].rearrange("p (t k b) -> p t k b", k=K2, b=BL)
            for t in range(T):
                if t == TH:
                    tensor.wait_ge(sem_precp, 2 * M_TILES)  # XW half 1
                if t >= 2:
                    # ps_z bank reuse: ACT consumed step t-2 gates
                    tensor.wait_ge(sem_act, 3 * (t - 2) + 2)
                # prefill z with XW[t] in one identity matmul BEFORE waiting
                # for h: it only depends on XW, so it runs during the tail
                # of step t-1 (and keeps PE a little warmer).
                tensor.matmul(ps_z[t % 2][:, 0:128], sb_I[:],
                              sb_XW[:, t * 128:(t + 1) * 128],
                              start=True, stop=True, skip_group_check=True)
                tensor.wait_ge(sem_h, t + 1)
                for m in range(M_TILES):
                    for k in range(K2):
                        mm = tensor.matmul(
                            ps_z[t % 2][:, m * BL:(m + 1) * BL],
                            sb_WrT[:, k * 1024 + m * 128:k * 1024 + (m + 1) * 128],
                            sb_H[:, t * 32 + k * BL:t * 32 + (k + 1) * BL],
                            start=False, stop=False, skip_group_check=True,
                        )
                    if m == 5:
                        mm.then_inc(sem_pe)  # i,f,g columns complete
                mm.then_inc(sem_pe)          # o columns complete
                if t < M_TILES:
                    pre_mm(M_TILES + t)
                if t == TH:
                    # head for h_1..h_TH — all its inputs exist by now, and
                    # PE is otherwise idle during the tail of each step
                    for k in range(K2):
                        tensor.matmul(
                            ps_hd[0][0:11, 0:HALF],
                            sb_WoC[:, k * 11:(k + 1) * 11],
                            h4[:, 1:1 + TH, k, :],
                            start=(k == 0), stop=(k == 1),
                        ).then_inc(sem_hd)
            # ---- head, second half (h_{TH+1}..h_T) ----
            tensor.wait_ge(sem_h, T + 1)
            for k in range(K2):
                tensor.matmul(
                    ps_hd[1][0:11, 0:HALF],
                    sb_WoC[:, k * 11:(k + 1) * 11],
                    h4[:, 1 + TH:1 + 2 * TH, k, :],
                    start=(k == 0), stop=(k == 1),
                ).then_inc(sem_hd)

        @block.vector
        def _(vector):
            vector.memset(sb_H[:, 0:32], 0.0)
            vector.memset(sb_C[:, 0:32], 0.0).then_inc(sem_h)
            # ---- precompute copies: psum + b_lstm -> XW (fp16) ----
            vector.wait_ge(dma_bl, 16)
            xw4 = sb_XW[:].rearrange("p (t m b) -> p t m b", m=M_TILES, b=BL)

            def pre_copy(idx):
                half, m = divmod(idx, M_TILES)
                vector.wait_ge(sem_pre, idx + 1)
                psrc = ps_pre[idx % 2][:, 0:HALF].rearrange(
                    "p (t b) -> p t b", b=BL)
                nc.vector.tensor_scalar_add(
                    xw4[:, half * TH:(half + 1) * TH, m, :], psrc,
                    sb_blstm[:, m:m + 1],
                ).then_inc(sem_precp)

            for idx in range(M_TILES):
                pre_copy(idx)
            # same-engine fence: v2(0) reads C written by memset above
            vector.drain()
            # ---- scan pointwise ----
            Alu = mybir.AluOpType
            for t in range(T):
                s = t % 2
                gs = sb_G[:, s * 128:(s + 1) * 128]
                ss = ps_s[:, s * 32:(s + 1) * 32]
                # all gates arrive as tanh (i,f,o weight cols pre-halved on
                # host): v2 = (tf+1)(.)c = 2f(.)c ; u2 = (ti+1)(.)tg ; S = 2c'
                if t >= 1:
                    vector.wait_ge(sem_cv, t)  # c(t) committed (same engine)
                vector.wait_ge(sem_act, 3 * t + 1)
                nc.vector.scalar_tensor_tensor(
                    sb_Vt[:], gs[:, 32:64], 1.0, sb_C[:, t * 32:(t + 1) * 32],
                    Alu.add, Alu.mult)
                nc.vector.scalar_tensor_tensor(
                    sb_U[:], gs[:, 0:32], 1.0, gs[:, 64:96],
                    Alu.add, Alu.mult).then_inc(sem_uv)
                vector.wait_ge(sem_uv, t + 1)  # u/v committed (in-order pipe)
                nc.vector.tensor_add(ss, sb_U[:], sb_Vt[:]).then_inc(sem_cp)
                # true cell state for next step; hides under ACT tanh_c
                vector.wait_ge(sem_cp, t + 1)
                nc.vector.tensor_scalar_mul(
                    sb_C[:, (t + 1) * 32:(t + 2) * 32], ss, 0.5
                ).then_inc(sem_cv)
                # h2 = (to+1)(.)tanh(c') = 2h; h-consumers use halved weights
                vector.wait_ge(sem_act, 3 * t + 3)
                nc.vector.scalar_tensor_tensor(
                    sb_H[:, (t + 1) * 32:(t + 2) * 32], gs[:, 96:128], 1.0,
                    sb_TC[:, s * 32:(s + 1) * 32], Alu.add, Alu.mult
                ).then_inc(sem_h)
                if t < M_TILES:
                    pre_copy(M_TILES + t)
                if t == TH + 2:
                    vector.wait_ge(dma_bo, 16)
                    vector.wait_ge(sem_hd, K2)
                    nc.vector.tensor_scalar_add(
                        sb_head[:, 0:HALF], ps_hd[0][0:11, 0:HALF],
                        sb_bob[0:11, 0:1]).then_inc(sem_hdcp)
            # ---- head copy, second half ----
            vector.wait_ge(sem_hd, 2 * K2)
            nc.vector.tensor_scalar_add(
                sb_head[:, HALF:2 * HALF], ps_hd[1][0:11, 0:HALF],
                sb_bob[0:11, 0:1]).then_inc(sem_hdcp)

        @block.scalar
        def _(scalar):
            Tanh = mybir.ActivationFunctionType.Tanh
            for t in range(T):
                s = t % 2
                gs = sb_G[:, s * 128:(s + 1) * 128]
                scalar.wait_ge(sem_pe, 2 * t + 1)
                scalar.activation(gs[:, 0:96], ps_z[s][:, 0:96], Tanh
                                  ).then_inc(sem_act)
                scalar.wait_ge(sem_pe, 2 * t + 2)
                scalar.activation(gs[:, 96:128], ps_z[s][:, 96:128], Tanh
                                  ).then_inc(sem_act)
                scalar.wait_ge(sem_cp, t + 1)
                scalar.activation(sb_TC[:, s * 32:(s + 1) * 32],
                                  ps_s[:, s * 32:(s + 1) * 32], Tanh,
                                  scale=0.5).then_inc(sem_act)

    return nc, ctx


_BUILD_CACHE = {}


def _get_nc(T):
    if T not in _BUILD_CACHE:
        _BUILD_CACHE[T] = _build(T)
    return _BUILD_CACHE[T][0]


def _prep_inputs(X, u, Wk, Wr, b_lstm, Wo, bo, Wc, bc, T):
    """Build the 8 per-core input maps (numpy, host-side sharding)."""
    # column scaling: i,f,o gates get 0.5 (sigma(x) = (tanh(x/2)+1)/2);
    # row scaling: recurrent/head weights get 0.5 because h is stored as 2h.
    col_scale = np.ones((1, 1024), np.float32)
    col_scale[:, :512] = 0.5          # i, f
    col_scale[:, 768:] = 0.5          # o   (g stays unscaled)
    Wk_p = np.ascontiguousarray(Wk[:, GATE_PERM] * col_scale
                                ).astype(np.float16)
    Wr_p = (Wr[:, GATE_PERM].astype(np.float32) * col_scale) * 0.5
    WrT = np.ascontiguousarray(
        Wr_p.reshape(2, 128, 1024).transpose(1, 0, 2).reshape(128, 2048)
    ).astype(np.float16)
    blstm = np.ascontiguousarray(
        (b_lstm[GATE_PERM].astype(np.float32) * col_scale[0]
         ).reshape(8, 128).T)
    WoC = np.concatenate([Wo.astype(np.float32),
                          Wc[:256].astype(np.float32)], axis=1) * 0.5
    WoC = np.ascontiguousarray(
        WoC.reshape(2, 128, 11).transpose(1, 0, 2).reshape(128, 22)
    ).astype(np.float16)
    bob = np.concatenate([bo.astype(np.float32), [0.0]]).reshape(11, 1)
    bob = np.ascontiguousarray(bob, np.float32)

    ident = np.eye(128, dtype=np.float16)
    in_maps = []
    for i in range(NCORES):
        bsl = slice(i * BL, (i + 1) * BL)
        Xt = np.ascontiguousarray(
            X[bsl, :T, :].astype(np.float32).transpose(2, 1, 0)
            .reshape(128, T * BL)).astype(np.float16)
        in_maps.append({
            "Xt": Xt, "WkT": Wk_p, "WrT": WrT, "blstm": blstm,
            "WoC": WoC, "bob": bob, "ident": ident,
        })
    return in_maps


def _sigmoid64(x):
    return 1.0 / (1.0 + np.exp(-x.astype(np.float64)))


def _softmax32(x):
    x = x.astype(np.float32)
    e = np.exp(x - x.max(axis=-1, keepdims=True))
    return (e / e.sum(axis=-1, keepdims=True)).astype(np.float32)


def _fallback_scan(x_seq, u_seq, h0, c0, t0, Wk, Wr, b_lstm, Wo, bo, Wc, bc):
    """Continue the reference recurrence on host for one sample that did not
    halt by t0.  Returns the sample's output row (float32)."""
    h = h0.astype(np.float32).copy()
    c = c0.astype(np.float32).copy()
    Wk = Wk.astype(np.float32); Wr = Wr.astype(np.float32)
    b_lstm = b_lstm.astype(np.float32)
    sig = lambda v: 1.0 / (1.0 + np.exp(-v))
    Tt = x_seq.shape[0]
    logits_last = None
    for t in range(t0, Tt):
        z = x_seq[t] @ Wk + h @ Wr + b_lstm
        i, f, g, o = np.split(z, 4)
        i = sig(i); f = sig(f); g = np.tanh(g); o = sig(o)
        c = f * c + i * g
        h = o * np.tanh(c)
        y = h @ Wo.astype(np.float32) + bo.astype(np.float32)
        logits = _softmax32(y)
        pre = float(h @ Wc[:256, 0].astype(np.float32)) \
            + t * float(Wc[256, 0]) + float(bc[0])
        probs = (1.0 - EPS) * sig(np.float32(pre)) + EPS * 0.05
        if u_seq[t] < probs:
            return logits
        logits_last = logits
    return logits_last


def kernel(**inputs):
    X = np.asarray(inputs["X"], np.float32)
    u = np.asarray(inputs["u"], np.float32)
    Wk = np.asarray(inputs["Wk"], np.float32)
    Wr = np.asarray(inputs["Wr"], np.float32)
    b_lstm = np.asarray(inputs["b_lstm"], np.float32)
    Wo = np.asarray(inputs["Wo"], np.float32)
    bo = np.asarray(inputs["bo"], np.float32)
    Wc = np.asarray(inputs["Wc"], np.float32)
    bc = np.asarray(inputs["bc"], np.float32)
    T = T_EFF

    nc = _get_nc(T)
    in_maps = _prep_inputs(X, u, Wk, Wr, b_lstm, Wo, bo, Wc, bc, T)
    res = run_bass_kernel_spmd(nc, in_maps, list(range(NCORES)))

    wc_t = float(Wc[256, 0])
    bias_c = float(bc[0])
    tvec = np.arange(T, dtype=np.float64)

    out = np.zeros((B, C), np.float32)
    for i in range(NCORES):
        bsl = slice(i * BL, (i + 1) * BL)
        head = res.results[i]["head"]          # [11, T*BL]
        y_pre = head[0:10].reshape(10, T, BL).transpose(1, 2, 0)  # [T, b, 10]
        pre_c = head[10].reshape(T, BL).astype(np.float64)        # [T, b]
        probs = (1.0 - EPS) * _sigmoid64(pre_c + tvec[:, None] * wc_t + bias_c) \
            + EPS * 0.05
        u_core = u[bsl, :T, 0]                 # [b, T]
        a = u_core.T.astype(np.float64) < probs  # [T, b]
        halted = a.any(axis=0)
        tstar = np.argmax(a, axis=0)           # first halt step per sample
        logits = _softmax32(y_pre)             # [T, b, 10]
        for b_ in range(BL):
            if halted[b_]:
                out[i * BL + b_] = logits[tstar[b_], b_]
            else:
                sh = res.results[i]["state_h"].astype(np.float32) * 0.5
                sc = res.results[i]["state_c"].astype(np.float32)
                h_T = sh.reshape(128, 2, BL).transpose(2, 1, 0) \
                    .reshape(BL, 256)[b_]
                c_T = sc.reshape(128, 2, BL).transpose(2, 1, 0) \
                    .reshape(BL, 256)[b_]
                out[i * BL + b_] = _fallback_scan(
                    X[i * BL + b_], u[i * BL + b_, :, 0], h_T, c_T, T,
                    Wk, Wr, b_lstm, Wo, bo, Wc, bc)
    return out
